# revision 10
# baseline (speedup 1.0000x reference)
"""Trainium2 Bass kernel for CanonCausalMultiheadAttn (v2).

Sharding: tensor-parallel over heads across 8 cores (2 q-heads + 1 kv-head
per core), both batches replicated. Each core computes its heads' attention
for both batches; two AllToAlls (one per local q-head) exchange attention
outputs so each core owns one (batch, seq-slice) of the final output
projection.

v2 structure (vs v1):
  - Phase A fuses QKV proj + canon conv + qk-rmsnorm + rope per 512-seq
    chunk so DVE/Pool/Scalar work overlaps the PE matmuls and the PE never
    idles (keeps the PE p-state at full clock).  V's canon runs on the Pool
    (gpsimd) engine; x^2 and rsqrt run on the Scalar engine (single act
    table: {Copy, Square, Rsqrt}).
  - Phase B attention keeps scores in [sk, q] layout but computes PV as
    out[dh, q] = sum_i va_i^T @ P_i (va as stationary operand), which lands
    directly in the AllToAll layout - no per-block DMA transposes and no
    [128,1] reciprocals.  Row sums come from ones-column matmuls; the
    1/rowsum is broadcast via a rank-1 matmul and applied with one DVE mul.
    A lag-2 software pipeline hides the exp (Scalar) latency.
  - The AllToAll is split per local q-head: cc_h0 fires halfway through
    attention; cc_h1 is hidden behind the even-channel half of the output
    projection (split by channel-block parity).
"""
import sys

sys.path.insert(0, '/opt/trn_rl_repo')

import numpy as np
import ml_dtypes

import concourse.bass as bass
import concourse.mybir as mybir
import concourse.tile as tile
from concourse import bacc
from concourse.bass_utils import run_bass_kernel_spmd

F32 = mybir.dt.float32
F32R = mybir.dt.float32r
BF16 = mybir.dt.bfloat16
AF = mybir.ActivationFunctionType
ALU = mybir.AluOpType

B, S, D = 2, 2048, 2048
NH, NKV, DH = 16, 8, 128
K_CONV = 4
EPS = 1e-6
SCALE = 1.0 / float(np.sqrt(DH))
NEG = -1e9
N_CORES = 8
N_CHUNKS = S // 512     # 512-wide seq chunks per batch
N_SKB = S // 128        # 128-wide sk blocks per batch


def _build():
    nc = bacc.Bacc("TRN2", target_bir_lowering=False, debug=False,
                   num_devices=N_CORES)

    hsT = nc.dram_tensor("hsT", [D, B * S], BF16, kind="ExternalInput")
    wT = nc.dram_tensor("wT", [D, 512], BF16, kind="ExternalInput")
    woT = nc.dram_tensor("woT", [D, D], BF16, kind="ExternalInput")
    cw = nc.dram_tensor("cw", [512, K_CONV], F32, kind="ExternalInput")
    ropeAq = nc.dram_tensor("ropeAq", [DH, S], BF16, kind="ExternalInput")
    ropeBq = nc.dram_tensor("ropeBq", [DH, S], BF16, kind="ExternalInput")
    ropeAk = nc.dram_tensor("ropeAk", [DH, S], BF16, kind="ExternalInput")
    ropeBk = nc.dram_tensor("ropeBk", [DH, S], BF16, kind="ExternalInput")
    maskd = nc.dram_tensor("maskd", [128, 128], F32, kind="ExternalInput")
    out = nc.dram_tensor("out", [512, D], F32, kind="ExternalOutput")
    import os
    DEBUG = os.environ.get("KDBG", "") == "1"
    dbg = {}
    if DEBUG:
        dbg["roped00"] = nc.dram_tensor("d_roped00", [128, S], BF16, kind="ExternalOutput")
        dbg["roped02"] = nc.dram_tensor("d_roped02", [128, S], BF16, kind="ExternalOutput")
        dbg["vaug0"] = nc.dram_tensor("d_vaug0", [128, N_SKB * 128], BF16, kind="ExternalOutput")
        dbg["rkt0"] = nc.dram_tensor("d_rkt0", [128, N_SKB], F32, kind="ExternalOutput")
        dbg["pt00"] = nc.dram_tensor("d_pt00", [128, 512], BF16, kind="ExternalOutput")
        dbg["rs00"] = nc.dram_tensor("d_rs00", [1, 512], F32, kind="ExternalOutput")
        dbg["oa00"] = nc.dram_tensor("d_oa00", [128, 512], BF16, kind="ExternalOutput")
        dbg["a2ao0"] = nc.dram_tensor("d_a2ao0", [N_CORES * 128, 512], BF16, kind="ExternalOutput")

    with tile.TileContext(nc) as tc:
        with tc.tile_pool(name="const", bufs=1) as cpool, \
             tc.tile_pool(name="persist", bufs=1) as pers, \
             tc.tile_pool(name="dram", bufs=1, space="DRAM") as dram:

            # ---- constants (weights first so QKV can start ASAP) ----
            w_sb = cpool.tile([128, 16 * 512], BF16, tag="wsb", name="w_sb")
            nc.sync.dma_start(
                w_sb[:].rearrange("p (k s) -> p k s", s=512),
                wT.ap().rearrange("(k p) s -> p k s", p=128))
            wv = w_sb[:].rearrange("p (k s) -> p k s", s=512)
            mask_sb = cpool.tile([128, 128], F32, tag="mask")
            nc.sync.dma_start(mask_sb[:], maskd.ap())
            ropes = {}
            for nm, t in (("Aq", ropeAq), ("Bq", ropeBq),
                          ("Ak", ropeAk), ("Bk", ropeBk)):
                rt = cpool.tile([DH, S], BF16, tag=f"rope{nm}", name=f"rope{nm}")
                nc.sync.dma_start(rt[:], t.ap())
                ropes[nm] = rt
            cw_sb = []
            for mt in range(4):
                t = cpool.tile([128, K_CONV], F32, tag=f"cw{mt}", name=f"cw{mt}")
                nc.sync.dma_start(t[:], cw.ap()[128 * mt:128 * mt + 128, :])
                cw_sb.append(t)
            ones_col = cpool.tile([128, 1], BF16, tag="oc")
            nc.vector.memset(ones_col[:], 1.0)
            eps_sb = cpool.tile([1, 1], F32, tag="eps")
            nc.vector.memset(eps_sb[:], EPS)
            ones_row_f = cpool.tile([1, 128], F32, tag="orf")
            nc.vector.memset(ones_row_f[:], 1.0)
            ones_row = cpool.tile([1, 128], F32R, tag="or")
            nc.scalar.copy(ones_row[:], ones_row_f[:])
            s0_sb = []
            for mt in range(4):
                t = cpool.tile([128, 1], F32, tag=f"s0{mt}", name=f"s0{mt}")
                nc.vector.tensor_scalar_add(t[:], cw_sb[mt][:, 0:1], 1.0)
                s0_sb.append(t)

            # persistent per-(b,mt) tiles
            roped = {}   # (b, mt<3) -> [128, S] bf16 (q0,q1 scaled by rstd)
            vaug = {}    # b -> [128, N_SKB*128] bf16 (transposed V)
            rstdkT = {}  # b -> [128, N_SKB] f32 (k rstd, transposed)

            for b in range(B):
                vaug[b] = pers.tile([128, N_SKB * 128], BF16, tag=f"vaug{b}",
                                    name=f"vaug{b}")
                rstdkT[b] = pers.tile([128, N_SKB], F32, tag=f"rstdkT{b}",
                                      name=f"rstdkT{b}")
                for mt in range(3):
                    roped[(b, mt)] = pers.tile([128, S], BF16,
                                               tag=f"roped{b}{mt}",
                                               name=f"roped{b}{mt}")

            # ============ phase A: QKV + canon + norm + rope ============
            # Two-stage pipeline: stage1(n) = hs DMA + QKV matmuls + psum->bf16
            # copies; stage2(n) = canon + norm + rope.  stage2(n-1) is emitted
            # after stage1(n) so its colsum/bcast matmuls never stall the PE.
            with tc.tile_pool(name="awork", bufs=1) as bw, \
                 tc.tile_pool(name="qps", bufs=1, space="PSUM") as qps, \
                 tc.tile_pool(name="nps", bufs=2, space="PSUM") as nps, \
                 tc.tile_pool(name="bps", bufs=1, space="PSUM") as bps:
                for b in range(B):
                    rk_d = dram.tile([N_SKB, 128], F32, tag=f"rkd{b}",
                                     name=f"rk_d{b}")

                    def stage1(n, b=b):
                        lo = 512 * n
                        hsp = []
                        for q4 in range(4):
                            t = bw.tile([128, 4 * 512], BF16, tag=f"hs{q4}",
                                        bufs=2, name=f"hs{q4}")
                            nc.sync.dma_start(
                                t[:].rearrange("p (k s) -> p k s", s=512),
                                hsT.ap()[512 * q4:512 * (q4 + 1),
                                         b * S + lo:b * S + lo + 512]
                                .rearrange("(k p) s -> p k s", p=128))
                            hsp.append(t[:].rearrange("p (k s) -> p k s",
                                                      s=512))
                        psums = [qps.tile([128, 512], F32, tag=f"qk{mt}",
                                          name=f"qk{mt}") for mt in range(4)]
                        for k in range(16):
                            for mt in range(4):
                                nc.tensor.matmul(
                                    psums[mt][:],
                                    wv[:, k, 128 * mt:128 * (mt + 1)],
                                    hsp[k // 4][:, k % 4, :],
                                    start=(k == 0), stop=(k == 15))
                        raws = []
                        for mt in range(4):
                            raw_c = bw.tile([128, 512], BF16, tag=f"rawc{mt}",
                                            bufs=3, name=f"rawc{mt}")
                            nc.scalar.copy(raw_c[:], psums[mt][:])
                            raws.append(raw_c)
                        return raws

                    def stage2(n, raws, prev, b=b):
                        lo = 512 * n
                        cn = {}
                        for mt in range(4):
                            c = bw.tile([128, 512], BF16, tag=f"cn{mt}",
                                        bufs=2, name=f"cn{mt}")
                            nc.vector.tensor_scalar_mul(c[:], raws[mt][:],
                                                        s0_sb[mt][:])
                            for k in range(1, K_CONV):
                                nc.vector.scalar_tensor_tensor(
                                    c[:, k:512], raws[mt][:, 0:512 - k],
                                    cw_sb[mt][:, k:k + 1], c[:, k:512],
                                    ALU.mult, ALU.add)
                                if prev is not None:
                                    nc.vector.scalar_tensor_tensor(
                                        c[:, 0:k],
                                        prev[mt][:, 512 - k:512],
                                        cw_sb[mt][:, k:k + 1], c[:, 0:k],
                                        ALU.mult, ALU.add)
                            cn[mt] = c
                        for i in range(4):
                            nc.sync.dma_start_transpose(
                                vaug[b][:, 128 * (4 * n + i):
                                        128 * (4 * n + i + 1)],
                                cn[3][:, 128 * i:128 * (i + 1)])
                        # rmsnorm rstd on the scalar engine:
                        # rstd = exp(-0.5*ln(meansq + eps))
                        rstd = {}
                        for mt in range(3):
                            sq = bw.tile([128, 512], BF16, tag="sq", bufs=2,
                                         name="sq")
                            nc.scalar.activation(sq[:], cn[mt][:], AF.Square)
                            sp = nps.tile([1, 512], F32, tag="ssq")
                            nc.tensor.matmul(sp[:], ones_col[:], sq[:],
                                             start=True, stop=True)
                            rs = bw.tile([1, 512], F32R if mt < 2 else F32,
                                         tag=f"rstd{mt}", bufs=2,
                                         name=f"rstd{mt}")
                            with nc.allow_low_precision(
                                    reason="rstd f32r is plenty"):
                                nc.scalar.activation(rs[:], sp[:],
                                                     AF.Abs_reciprocal_sqrt,
                                                     bias=eps_sb[:],
                                                     scale=1.0 / DH)
                            rstd[mt] = rs
                        nc.sync.dma_start(rk_d[4 * n:4 * (n + 1), :],
                                          rstd[2][:])
                        for mt in range(3):
                            is_q = mt < 2
                            A_ = ropes["Aq"] if is_q else ropes["Ak"]
                            B_ = ropes["Bq"] if is_q else ropes["Bk"]
                            c = cn[mt]
                            sh = bw.tile([128, 512], BF16, tag="sh", bufs=2,
                                         name="sh")
                            nc.sync.dma_start(sh[0:64, :], c[64:128, :])
                            nc.sync.dma_start(sh[64:128, :], c[0:64, :])
                            t1 = bw.tile([128, 512], BF16, tag="t1", bufs=2,
                                         name="t1")
                            nc.vector.tensor_mul(t1[:], sh[:],
                                                 B_[:, lo:lo + 512])
                            t2 = bw.tile([128, 512], BF16, tag="t2", bufs=2,
                                         name="t2")
                            nc.vector.tensor_mul(t2[:], c[:],
                                                 A_[:, lo:lo + 512])
                            ro = roped[(b, mt)]
                            if is_q:
                                bc = bps.tile([128, 512], F32, tag=f"bc{mt}",
                                              name=f"bc{mt}")
                                nc.tensor.matmul(bc[:], ones_row[:],
                                                 rstd[mt][:], start=True,
                                                 stop=True)
                                bcb = bw.tile([128, 512], BF16,
                                              tag="bcb", bufs=2, name="bcb")
                                nc.scalar.copy(bcb[:], bc[:])
                                t3 = bw.tile([128, 512], BF16, tag="t3",
                                             bufs=2, name="t3")
                                nc.vector.tensor_add(t3[:], t1[:], t2[:])
                                nc.vector.tensor_mul(ro[:, lo:lo + 512],
                                                     t3[:], bcb[:])
                            else:
                                nc.vector.tensor_add(ro[:, lo:lo + 512],
                                                     t1[:], t2[:])

                    prev = None
                    pend_st2 = None
                    for n in range(N_CHUNKS):
                        raws = stage1(n)
                        if pend_st2 is not None:
                            stage2(n - 1, pend_st2, prev)
                            prev = pend_st2
                        pend_st2 = raws
                    stage2(N_CHUNKS - 1, pend_st2, prev)
                    nc.sync.dma_start(rstdkT[b][:],
                                      rk_d[:].rearrange("i p -> p i"))
                    if DEBUG and b == 0:
                        nc.sync.dma_start(dbg["roped00"].ap(), roped[(0, 0)][:])
                        nc.sync.dma_start(dbg["roped02"].ap(), roped[(0, 2)][:])
                        nc.sync.dma_start(dbg["vaug0"].ap(), vaug[0][:])
                        nc.sync.dma_start(dbg["rkt0"].ap(), rstdkT[0][:])

            # ======================= attention =======================
            # a2a buffers: one per local q-head, [8*128 rows, 512 cols]
            a2a_in = [dram.tile([N_CORES * 128, 512], BF16, tag=f"a2ai{h}",
                                name=f"a2a_in{h}") for h in range(2)]
            a2a_out = [dram.tile([N_CORES * 128, 512], BF16, tag=f"a2ao{h}",
                                 name=f"a2a_out{h}") for h in range(2)]

            with tc.tile_pool(name="scps", bufs=4, space="PSUM") as scps, \
                 tc.tile_pool(name="pvps", bufs=2, space="PSUM") as pvps, \
                 tc.tile_pool(name="rsps", bufs=1, space="PSUM") as rsps, \
                 tc.tile_pool(name="abps", bufs=1, space="PSUM") as abps, \
                 tc.tile_pool(name="apool", bufs=1) as apool:
                for h in range(2):
                    for b in range(B):
                        KT = roped[(b, 2)]
                        QT = roped[(b, h)]
                        va = vaug[b]
                        rkt = rstdkT[b]
                        for j in range(N_CHUNKS):
                            nij = 4 * j + 4
                            out_ps = pvps.tile([128, 512], F32, tag="pv",
                                               name="out_ps")
                            rs_ps = rsps.tile([1, 512], F32, tag="rs",
                                              name="rs_ps")

                            def emit_pv(i, pt):
                                nc.tensor.matmul(
                                    out_ps[:],
                                    va[:, 128 * i:128 * (i + 1)], pt[:],
                                    start=(i == 0), stop=(i == nij - 1))
                                nc.tensor.matmul(
                                    rs_ps[:], ones_col[:], pt[:],
                                    start=(i == 0), stop=(i == nij - 1))

                            pend = []
                            for i in range(nij):
                                r = i - 4 * j
                                off = 128 * max(r, 0)
                                sc = scps.tile([128, 512], F32, tag="sc",
                                               name="sc")
                                nc.tensor.matmul(
                                    sc[:, off:512],
                                    KT[:, 128 * i:128 * (i + 1)],
                                    QT[:, 512 * j + off:512 * (j + 1)],
                                    start=True, stop=True)
                                if r >= 0:
                                    nc.vector.tensor_add(
                                        sc[:, off:off + 128],
                                        sc[:, off:off + 128], mask_sb[:])
                                pt = apool.tile([128, 512], BF16, tag="pt",
                                                bufs=5, name="pt")
                                if off > 0:
                                    nc.vector.memset(pt[:, 0:off], 0.0)
                                nc.scalar.activation(
                                    pt[:, off:512], sc[:, off:512], AF.Exp,
                                    scale=rkt[:, i:i + 1])
                                if DEBUG and h == 0 and b == 0 and j == 0 and i == 0:
                                    nc.sync.dma_start(dbg["pt00"].ap(), pt[:])
                                pend.append((i, pt))
                                if len(pend) > 2:
                                    emit_pv(*pend.pop(0))
                            for it in pend:
                                emit_pv(*it)
                            # normalize and ship
                            rr = apool.tile([1, 512], F32R, tag="rr", bufs=2,
                                            name="rr")
                            with nc.allow_low_precision(
                                    reason="softmax denom f32r ample"):
                                nc.vector.reciprocal(rr[:], rs_ps[:])
                            bcn = abps.tile([128, 512], F32, tag="bcn",
                                            name="bcn")
                            nc.tensor.matmul(bcn[:], ones_row[:], rr[:],
                                             start=True, stop=True)
                            bcs = apool.tile([128, 512], F32, tag="bcs",
                                             bufs=2, name="bcs")
                            nc.vector.tensor_copy(bcs[:], bcn[:])
                            oa = apool.tile([128, 512], BF16, tag="oa",
                                            bufs=2, name="oa")
                            nc.vector.tensor_mul(oa[:], out_ps[:], bcs[:])
                            rd = 4 * b + j
                            if DEBUG and h == 0 and b == 0 and j == 0:
                                nc.sync.dma_start(dbg["oa00"].ap(), oa[:])
                            nc.sync.dma_start(
                                a2a_in[h][128 * rd:128 * (rd + 1), :], oa[:])
                    # per-head AllToAll as soon as this head's blocks done
                    nc.gpsimd.collective_compute(
                        "AllToAll", ALU.bypass,
                        replica_groups=[list(range(N_CORES))],
                        ins=[a2a_in[h].opt()], outs=[a2a_out[h].opt()],
                        cc_dim="Partition")

            # ====================== out projection ====================
            # channel block 2*s+h of the gathered activation lives in
            # a2a_out[h] src-block s.  Split the contraction by h so the
            # h=1 AllToAll hides behind the h=0 half of the matmul.
            with tc.tile_pool(name="opool", bufs=1) as opool, \
                 tc.tile_pool(name="ops", bufs=1, space="PSUM") as ops:
                aout = []
                for h in range(2):
                    ao = opool.tile([128, 8 * 512], BF16, tag=f"ao{h}",
                                    name=f"ao{h}")
                    nc.sync.dma_start(
                        ao[:].rearrange("p (k s) -> p k s", s=512),
                        a2a_out[h][:].rearrange("(k p) s -> p k s", p=128))
                    aout.append(ao[:].rearrange("p (k s) -> p k s", s=512))
                if DEBUG:
                    nc.sync.dma_start(dbg["a2ao0"].ap(), a2a_out[0][:])
                for np_ in range(2):
                    pso = {}
                    for mp in range(4):
                        for nn in range(2):
                            pso[(mp, nn)] = ops.tile(
                                [128, 512], F32, tag=f"o{mp}{nn}",
                                name=f"o{mp}{nn}")
                    for h in range(2):
                        for k8 in range(8):
                            kb = 2 * k8 + h
                            wo_t = opool.tile([128, 1024], BF16, tag="wo",
                                              bufs=4, name="wo_t")
                            nc.sync.dma_start(
                                wo_t[:],
                                woT.ap()[128 * kb:128 * (kb + 1),
                                         1024 * np_:1024 * (np_ + 1)])
                            for mp in range(4):
                                for nn in range(2):
                                    nc.tensor.matmul(
                                        pso[(mp, nn)][:],
                                        aout[h][:, k8,
                                                128 * mp:128 * (mp + 1)],
                                        wo_t[:, 512 * nn:512 * (nn + 1)],
                                        start=(h == 0 and k8 == 0),
                                        stop=(h == 1 and k8 == 7))
                    for mp in range(4):
                        for nn in range(2):
                            os_t = opool.tile([128, 512], F32, tag="osb",
                                              bufs=4, name="os_t")
                            nc.scalar.copy(os_t[:], pso[(mp, nn)][:])
                            nc.sync.dma_start(
                                out.ap()[128 * mp:128 * (mp + 1),
                                         1024 * np_ + 512 * nn:
                                         1024 * np_ + 512 * (nn + 1)],
                                os_t[:])

    nc.compile()
    return nc


_NC_CACHE = None


def _get_nc():
    global _NC_CACHE
    if _NC_CACHE is None:
        _NC_CACHE = _build()
    return _NC_CACHE


def _host_prep(inputs):
    hs = np.asarray(inputs["hidden_states"], dtype=np.float32)
    Wq = np.asarray(inputs["Wq"], dtype=np.float32)
    Wk = np.asarray(inputs["Wk"], dtype=np.float32)
    Wv = np.asarray(inputs["Wv"], dtype=np.float32)
    Wo = np.asarray(inputs["Wo"], dtype=np.float32)
    cqw = np.asarray(inputs["canon_q_w"], dtype=np.float32)
    ckw = np.asarray(inputs["canon_k_w"], dtype=np.float32)
    cvw = np.asarray(inputs["canon_v_w"], dtype=np.float32)
    qnw = np.asarray(inputs["q_norm_w"], dtype=np.float32)
    knw = np.asarray(inputs["k_norm_w"], dtype=np.float32)

    bf = ml_dtypes.bfloat16
    hsT = np.ascontiguousarray(
        np.concatenate([hs[0].T, hs[1].T], axis=1)).astype(bf)
    WqT, WkT, WvT = Wq.T, Wk.T, Wv.T
    woT = np.ascontiguousarray(Wo.T).astype(bf)

    inv_freq = 1.0 / (10000.0 ** (np.arange(0, DH, 2, dtype=np.float64) / DH))
    freqs = np.arange(S, dtype=np.float64)[:, None] * inv_freq
    emb = np.concatenate([freqs, freqs], axis=-1)
    cosT, sinT = np.cos(emb).T, np.sin(emb).T

    def make_rope(normw, scale):
        A = cosT * normw[:, None] * scale
        wswap = normw[(np.arange(DH) + 64) % DH]
        sign = np.where(np.arange(DH) < 64, -1.0, 1.0)
        Bc = sinT * wswap[:, None] * sign[:, None] * scale
        return (np.ascontiguousarray(A).astype(bf),
                np.ascontiguousarray(Bc).astype(bf))

    Aq, Bq = make_rope(qnw, SCALE)
    Ak, Bk = make_rope(knw, 1.0)

    p = np.arange(128)[:, None]
    f = np.arange(128)[None, :]
    maskd = np.where(p <= f, 0.0, NEG).astype(np.float32)

    in_maps = []
    for r in range(N_CORES):
        wTc = np.ascontiguousarray(np.concatenate(
            [WqT[:, 256 * r:256 * r + 256],
             WkT[:, 128 * r:128 * r + 128],
             WvT[:, 128 * r:128 * r + 128]], axis=1)).astype(bf)
        cwc = np.ascontiguousarray(np.concatenate(
            [cqw[256 * r:256 * r + 256],
             ckw[128 * r:128 * r + 128],
             cvw[128 * r:128 * r + 128]], axis=0)).astype(np.float32)
        in_maps.append({
            "hsT": hsT, "wT": wTc, "woT": woT, "cw": cwc,
            "ropeAq": Aq, "ropeBq": Bq, "ropeAk": Ak, "ropeBk": Bk,
            "maskd": maskd,
        })
    return in_maps


def kernel(**inputs):
    nc = _get_nc()
    in_maps = _host_prep(inputs)
    res = run_bass_kernel_spmd(nc, in_maps, core_ids=list(range(N_CORES)))
    full = np.empty((B, S, D), np.float32)
    for r in range(N_CORES):
        full[r // 4, 512 * (r % 4):512 * (r % 4 + 1), :] = res.results[r]["out"]
    return full


# revision 11
# speedup vs baseline: 1.0022x; 1.0022x over previous
"""Trainium2 Bass kernel for CanonCausalMultiheadAttn (v2).

Sharding: tensor-parallel over heads across 8 cores (2 q-heads + 1 kv-head
per core), both batches replicated. Each core computes its heads' attention
for both batches; two AllToAlls (one per local q-head) exchange attention
outputs so each core owns one (batch, seq-slice) of the final output
projection.

v2 structure (vs v1):
  - Phase A fuses QKV proj + canon conv + qk-rmsnorm + rope per 512-seq
    chunk so DVE/Pool/Scalar work overlaps the PE matmuls and the PE never
    idles (keeps the PE p-state at full clock).  V's canon runs on the Pool
    (gpsimd) engine; x^2 and rsqrt run on the Scalar engine (single act
    table: {Copy, Square, Rsqrt}).
  - Phase B attention keeps scores in [sk, q] layout but computes PV as
    out[dh, q] = sum_i va_i^T @ P_i (va as stationary operand), which lands
    directly in the AllToAll layout - no per-block DMA transposes and no
    [128,1] reciprocals.  Row sums come from ones-column matmuls; the
    1/rowsum is broadcast via a rank-1 matmul and applied with one DVE mul.
    A lag-2 software pipeline hides the exp (Scalar) latency.
  - The AllToAll is split per local q-head: cc_h0 fires halfway through
    attention; cc_h1 is hidden behind the even-channel half of the output
    projection (split by channel-block parity).
"""
import sys

sys.path.insert(0, '/opt/trn_rl_repo')

import numpy as np
import ml_dtypes

import concourse.bass as bass
import concourse.mybir as mybir
import concourse.tile as tile
from concourse import bacc
from concourse.bass_utils import run_bass_kernel_spmd

F32 = mybir.dt.float32
F32R = mybir.dt.float32r
BF16 = mybir.dt.bfloat16
AF = mybir.ActivationFunctionType
ALU = mybir.AluOpType

B, S, D = 2, 2048, 2048
NH, NKV, DH = 16, 8, 128
K_CONV = 4
EPS = 1e-6
SCALE = 1.0 / float(np.sqrt(DH))
NEG = -1e9
N_CORES = 8
N_CHUNKS = S // 512     # 512-wide seq chunks per batch
N_SKB = S // 128        # 128-wide sk blocks per batch


def _build():
    nc = bacc.Bacc("TRN2", target_bir_lowering=False, debug=False,
                   num_devices=N_CORES)

    hsT = nc.dram_tensor("hsT", [D, B * S], BF16, kind="ExternalInput")
    wT = nc.dram_tensor("wT", [D, 512], BF16, kind="ExternalInput")
    woT = nc.dram_tensor("woT", [D, D], BF16, kind="ExternalInput")
    cw = nc.dram_tensor("cw", [512, K_CONV], F32, kind="ExternalInput")
    ropeAq = nc.dram_tensor("ropeAq", [DH, S], BF16, kind="ExternalInput")
    ropeBq = nc.dram_tensor("ropeBq", [DH, S], BF16, kind="ExternalInput")
    ropeAk = nc.dram_tensor("ropeAk", [DH, S], BF16, kind="ExternalInput")
    ropeBk = nc.dram_tensor("ropeBk", [DH, S], BF16, kind="ExternalInput")
    maskd = nc.dram_tensor("maskd", [128, 128], F32, kind="ExternalInput")
    out = nc.dram_tensor("out", [512, D], F32, kind="ExternalOutput")
    import os
    DEBUG = os.environ.get("KDBG", "") == "1"
    dbg = {}
    if DEBUG:
        dbg["roped00"] = nc.dram_tensor("d_roped00", [128, S], BF16, kind="ExternalOutput")
        dbg["roped02"] = nc.dram_tensor("d_roped02", [128, S], BF16, kind="ExternalOutput")
        dbg["vaug0"] = nc.dram_tensor("d_vaug0", [128, N_SKB * 128], BF16, kind="ExternalOutput")
        dbg["rkt0"] = nc.dram_tensor("d_rkt0", [128, N_SKB], F32, kind="ExternalOutput")
        dbg["pt00"] = nc.dram_tensor("d_pt00", [128, 512], BF16, kind="ExternalOutput")
        dbg["rs00"] = nc.dram_tensor("d_rs00", [1, 512], F32, kind="ExternalOutput")
        dbg["oa00"] = nc.dram_tensor("d_oa00", [128, 512], BF16, kind="ExternalOutput")
        dbg["a2ao0"] = nc.dram_tensor("d_a2ao0", [N_CORES * 128, 512], BF16, kind="ExternalOutput")

    with tile.TileContext(nc) as tc:
        with tc.tile_pool(name="const", bufs=1) as cpool, \
             tc.tile_pool(name="persist", bufs=1) as pers, \
             tc.tile_pool(name="dram", bufs=1, space="DRAM") as dram:

            # ---- constants (weights first so QKV can start ASAP) ----
            wvq = []
            for q4 in range(4):
                t = cpool.tile([128, 4 * 512], BF16, tag=f"wsb{q4}",
                               name=f"wsb{q4}")
                nc.sync.dma_start(
                    t[:].rearrange("p (k s) -> p k s", s=512),
                    wT.ap()[512 * q4:512 * (q4 + 1), :]
                    .rearrange("(k p) s -> p k s", p=128))
                wvq.append(t[:].rearrange("p (k s) -> p k s", s=512))
            mask_sb = cpool.tile([128, 128], F32, tag="mask")
            nc.sync.dma_start(mask_sb[:], maskd.ap())
            ropes = {}
            for nm, t in (("Aq", ropeAq), ("Bq", ropeBq),
                          ("Ak", ropeAk), ("Bk", ropeBk)):
                rt = cpool.tile([DH, S], BF16, tag=f"rope{nm}", name=f"rope{nm}")
                nc.sync.dma_start(rt[:], t.ap())
                ropes[nm] = rt
            cw_sb = []
            for mt in range(4):
                t = cpool.tile([128, K_CONV], F32, tag=f"cw{mt}", name=f"cw{mt}")
                nc.sync.dma_start(t[:], cw.ap()[128 * mt:128 * mt + 128, :])
                cw_sb.append(t)
            ones_col = cpool.tile([128, 1], BF16, tag="oc")
            nc.vector.memset(ones_col[:], 1.0)
            eps_sb = cpool.tile([1, 1], F32, tag="eps")
            nc.vector.memset(eps_sb[:], EPS)
            ones_row_f = cpool.tile([1, 128], F32, tag="orf")
            nc.vector.memset(ones_row_f[:], 1.0)
            ones_row = cpool.tile([1, 128], F32R, tag="or")
            nc.scalar.copy(ones_row[:], ones_row_f[:])
            s0_sb = []
            for mt in range(4):
                t = cpool.tile([128, 1], F32, tag=f"s0{mt}", name=f"s0{mt}")
                nc.vector.tensor_scalar_add(t[:], cw_sb[mt][:, 0:1], 1.0)
                s0_sb.append(t)

            # persistent per-(b,mt) tiles
            roped = {}   # (b, mt<3) -> [128, S] bf16 (q0,q1 scaled by rstd)
            vaug = {}    # b -> [128, N_SKB*128] bf16 (transposed V)
            rstdkT = {}  # b -> [128, N_SKB] f32 (k rstd, transposed)

            for b in range(B):
                vaug[b] = pers.tile([128, N_SKB * 128], BF16, tag=f"vaug{b}",
                                    name=f"vaug{b}")
                rstdkT[b] = pers.tile([128, N_SKB], F32, tag=f"rstdkT{b}",
                                      name=f"rstdkT{b}")
                for mt in range(3):
                    roped[(b, mt)] = pers.tile([128, S], BF16,
                                               tag=f"roped{b}{mt}",
                                               name=f"roped{b}{mt}")

            # ============ phase A: QKV + canon + norm + rope ============
            # Two-stage pipeline: stage1(n) = hs DMA + QKV matmuls + psum->bf16
            # copies; stage2(n) = canon + norm + rope.  stage2(n-1) is emitted
            # after stage1(n) so its colsum/bcast matmuls never stall the PE.
            with tc.tile_pool(name="awork", bufs=1) as bw, \
                 tc.tile_pool(name="qps", bufs=1, space="PSUM") as qps, \
                 tc.tile_pool(name="nps", bufs=2, space="PSUM") as nps, \
                 tc.tile_pool(name="bps", bufs=1, space="PSUM") as bps:
                rk_ds = {b: dram.tile([N_SKB, 128], F32, tag=f"rkd{b}",
                                      name=f"rk_d{b}") for b in range(B)}
                if True:

                    def stage1(n, b):
                        rk_d = rk_ds[b]
                        lo = 512 * n
                        hsp = []
                        for q4 in range(4):
                            t = bw.tile([128, 4 * 512], BF16, tag=f"hs{q4}",
                                        bufs=2, name=f"hs{q4}")
                            nc.sync.dma_start(
                                t[:].rearrange("p (k s) -> p k s", s=512),
                                hsT.ap()[512 * q4:512 * (q4 + 1),
                                         b * S + lo:b * S + lo + 512]
                                .rearrange("(k p) s -> p k s", p=128))
                            hsp.append(t[:].rearrange("p (k s) -> p k s",
                                                      s=512))
                        psums = [qps.tile([128, 512], F32, tag=f"qk{mt}",
                                          name=f"qk{mt}") for mt in range(4)]
                        for k in range(16):
                            for mt in range(4):
                                nc.tensor.matmul(
                                    psums[mt][:],
                                    wvq[k // 4][:, k % 4,
                                                128 * mt:128 * (mt + 1)],
                                    hsp[k // 4][:, k % 4, :],
                                    start=(k == 0), stop=(k == 15))
                        raws = []
                        for mt in range(4):
                            raw_c = bw.tile([128, 512], BF16, tag=f"rawc{mt}",
                                            bufs=3, name=f"rawc{mt}")
                            nc.scalar.copy(raw_c[:], psums[mt][:])
                            raws.append(raw_c)
                        return raws

                    def stage2(n, raws, prev, b):
                        rk_d = rk_ds[b]
                        lo = 512 * n
                        cn = {}
                        for mt in range(4):
                            c = bw.tile([128, 512], BF16, tag=f"cn{mt}",
                                        bufs=2, name=f"cn{mt}")
                            nc.vector.tensor_scalar_mul(c[:], raws[mt][:],
                                                        s0_sb[mt][:])
                            for k in range(1, K_CONV):
                                nc.vector.scalar_tensor_tensor(
                                    c[:, k:512], raws[mt][:, 0:512 - k],
                                    cw_sb[mt][:, k:k + 1], c[:, k:512],
                                    ALU.mult, ALU.add)
                                if prev is not None:
                                    nc.vector.scalar_tensor_tensor(
                                        c[:, 0:k],
                                        prev[mt][:, 512 - k:512],
                                        cw_sb[mt][:, k:k + 1], c[:, 0:k],
                                        ALU.mult, ALU.add)
                            cn[mt] = c
                        for i in range(4):
                            nc.sync.dma_start_transpose(
                                vaug[b][:, 128 * (4 * n + i):
                                        128 * (4 * n + i + 1)],
                                cn[3][:, 128 * i:128 * (i + 1)])
                        # rmsnorm rstd on the scalar engine:
                        # rstd = exp(-0.5*ln(meansq + eps))
                        rstd = {}
                        for mt in range(3):
                            sq = bw.tile([128, 512], BF16, tag="sq", bufs=2,
                                         name="sq")
                            nc.scalar.activation(sq[:], cn[mt][:], AF.Square)
                            sp = nps.tile([1, 512], F32, tag="ssq")
                            nc.tensor.matmul(sp[:], ones_col[:], sq[:],
                                             start=True, stop=True)
                            rs = bw.tile([1, 512], F32R if mt < 2 else F32,
                                         tag=f"rstd{mt}", bufs=2,
                                         name=f"rstd{mt}")
                            with nc.allow_low_precision(
                                    reason="rstd f32r is plenty"):
                                nc.scalar.activation(rs[:], sp[:],
                                                     AF.Abs_reciprocal_sqrt,
                                                     bias=eps_sb[:],
                                                     scale=1.0 / DH)
                            rstd[mt] = rs
                        nc.sync.dma_start(rk_d[4 * n:4 * (n + 1), :],
                                          rstd[2][:])
                        for mt in range(3):
                            is_q = mt < 2
                            A_ = ropes["Aq"] if is_q else ropes["Ak"]
                            B_ = ropes["Bq"] if is_q else ropes["Bk"]
                            c = cn[mt]
                            sh = bw.tile([128, 512], BF16, tag="sh", bufs=2,
                                         name="sh")
                            nc.sync.dma_start(sh[0:64, :], c[64:128, :])
                            nc.sync.dma_start(sh[64:128, :], c[0:64, :])
                            t1 = bw.tile([128, 512], BF16, tag="t1", bufs=2,
                                         name="t1")
                            nc.vector.tensor_mul(t1[:], sh[:],
                                                 B_[:, lo:lo + 512])
                            t2 = bw.tile([128, 512], BF16, tag="t2", bufs=2,
                                         name="t2")
                            nc.vector.tensor_mul(t2[:], c[:],
                                                 A_[:, lo:lo + 512])
                            ro = roped[(b, mt)]
                            if is_q:
                                bc = bps.tile([128, 512], F32, tag=f"bc{mt}",
                                              name=f"bc{mt}")
                                nc.tensor.matmul(bc[:], ones_row[:],
                                                 rstd[mt][:], start=True,
                                                 stop=True)
                                bcb = bw.tile([128, 512], BF16,
                                              tag="bcb", bufs=2, name="bcb")
                                nc.scalar.copy(bcb[:], bc[:])
                                t3 = bw.tile([128, 512], BF16, tag="t3",
                                             bufs=2, name="t3")
                                nc.vector.tensor_add(t3[:], t1[:], t2[:])
                                nc.vector.tensor_mul(ro[:, lo:lo + 512],
                                                     t3[:], bcb[:])
                            else:
                                nc.vector.tensor_add(ro[:, lo:lo + 512],
                                                     t1[:], t2[:])

                    def finish_batch(b):
                        nc.sync.dma_start(rstdkT[b][:],
                                          rk_ds[b][:].rearrange("i p -> p i"))
                        if DEBUG and b == 0:
                            nc.sync.dma_start(dbg["roped00"].ap(),
                                              roped[(0, 0)][:])
                            nc.sync.dma_start(dbg["roped02"].ap(),
                                              roped[(0, 2)][:])
                            nc.sync.dma_start(dbg["vaug0"].ap(), vaug[0][:])
                            nc.sync.dma_start(dbg["rkt0"].ap(), rstdkT[0][:])

                    pairs = [(b, n) for b in range(B) for n in range(N_CHUNKS)]
                    prev_by_b = {0: None, 1: None}
                    pend_st2 = None  # (b, n, raws)
                    for (b, n) in pairs:
                        raws = stage1(n, b)
                        if pend_st2 is not None:
                            pb, pn, praws = pend_st2
                            stage2(pn, praws, prev_by_b[pb], pb)
                            prev_by_b[pb] = praws
                            if pn == N_CHUNKS - 1:
                                finish_batch(pb)
                        pend_st2 = (b, n, raws)
                    pb, pn, praws = pend_st2
                    stage2(pn, praws, prev_by_b[pb], pb)
                    finish_batch(pb)

            # ======================= attention =======================
            # a2a buffers: one per local q-head, [8*128 rows, 512 cols]
            a2a_in = [dram.tile([N_CORES * 128, 512], BF16, tag=f"a2ai{h}",
                                name=f"a2a_in{h}") for h in range(2)]
            a2a_out = [dram.tile([N_CORES * 128, 512], BF16, tag=f"a2ao{h}",
                                 name=f"a2a_out{h}") for h in range(2)]

            with tc.tile_pool(name="scps", bufs=4, space="PSUM") as scps, \
                 tc.tile_pool(name="pvps", bufs=2, space="PSUM") as pvps, \
                 tc.tile_pool(name="rsps", bufs=1, space="PSUM") as rsps, \
                 tc.tile_pool(name="abps", bufs=1, space="PSUM") as abps, \
                 tc.tile_pool(name="apool", bufs=1) as apool:
                for h in range(2):
                    for b in range(B):
                        KT = roped[(b, 2)]
                        QT = roped[(b, h)]
                        va = vaug[b]
                        rkt = rstdkT[b]
                        for j in range(N_CHUNKS):
                            nij = 4 * j + 4
                            out_ps = pvps.tile([128, 512], F32, tag="pv",
                                               name="out_ps")
                            rs_ps = rsps.tile([1, 512], F32, tag="rs",
                                              name="rs_ps")

                            def emit_pv(i, pt):
                                nc.tensor.matmul(
                                    out_ps[:],
                                    va[:, 128 * i:128 * (i + 1)], pt[:],
                                    start=(i == 0), stop=(i == nij - 1))
                                nc.tensor.matmul(
                                    rs_ps[:], ones_col[:], pt[:],
                                    start=(i == 0), stop=(i == nij - 1))

                            pend = []
                            for i in range(nij):
                                r = i - 4 * j
                                off = 128 * max(r, 0)
                                sc = scps.tile([128, 512], F32, tag="sc",
                                               name="sc")
                                nc.tensor.matmul(
                                    sc[:, off:512],
                                    KT[:, 128 * i:128 * (i + 1)],
                                    QT[:, 512 * j + off:512 * (j + 1)],
                                    start=True, stop=True)
                                if r >= 0:
                                    nc.vector.tensor_add(
                                        sc[:, off:off + 128],
                                        sc[:, off:off + 128], mask_sb[:])
                                pt = apool.tile([128, 512], BF16, tag="pt",
                                                bufs=5, name="pt")
                                if off > 0:
                                    nc.vector.memset(pt[:, 0:off], 0.0)
                                nc.scalar.activation(
                                    pt[:, off:512], sc[:, off:512], AF.Exp,
                                    scale=rkt[:, i:i + 1])
                                if DEBUG and h == 0 and b == 0 and j == 0 and i == 0:
                                    nc.sync.dma_start(dbg["pt00"].ap(), pt[:])
                                pend.append((i, pt))
                                if len(pend) > 3:
                                    emit_pv(*pend.pop(0))
                            for it in pend:
                                emit_pv(*it)
                            # normalize and ship
                            rsb = apool.tile([1, 512], F32, tag="rsb",
                                             bufs=2, name="rsb")
                            nc.vector.tensor_copy(rsb[:], rs_ps[:])
                            rr = apool.tile([1, 512], F32R, tag="rr", bufs=2,
                                            name="rr")
                            with nc.allow_low_precision(
                                    reason="softmax denom f32r ample"):
                                nc.vector.reciprocal(rr[:], rsb[:])
                            bcn = abps.tile([128, 512], F32, tag="bcn",
                                            name="bcn")
                            nc.tensor.matmul(bcn[:], ones_row[:], rr[:],
                                             start=True, stop=True)
                            bcs = apool.tile([128, 512], F32, tag="bcs",
                                             bufs=2, name="bcs")
                            nc.vector.tensor_copy(bcs[:], bcn[:])
                            oa = apool.tile([128, 512], BF16, tag="oa",
                                            bufs=2, name="oa")
                            nc.vector.tensor_mul(oa[:], out_ps[:], bcs[:])
                            rd = 4 * b + j
                            if DEBUG and h == 0 and b == 0 and j == 0:
                                nc.sync.dma_start(dbg["oa00"].ap(), oa[:])
                            nc.sync.dma_start(
                                a2a_in[h][128 * rd:128 * (rd + 1), :], oa[:])
                    # per-head AllToAll as soon as this head's blocks done
                    nc.gpsimd.collective_compute(
                        "AllToAll", ALU.bypass,
                        replica_groups=[list(range(N_CORES))],
                        ins=[a2a_in[h].opt()], outs=[a2a_out[h].opt()],
                        cc_dim="Partition")

            # ====================== out projection ====================
            # channel block 2*s+h of the gathered activation lives in
            # a2a_out[h] src-block s.  Split the contraction by h so the
            # h=1 AllToAll hides behind the h=0 half of the matmul.
            with tc.tile_pool(name="opool", bufs=1) as opool, \
                 tc.tile_pool(name="ops", bufs=1, space="PSUM") as ops:
                aout = []
                for h in range(2):
                    ao = opool.tile([128, 8 * 512], BF16, tag=f"ao{h}",
                                    name=f"ao{h}")
                    nc.sync.dma_start(
                        ao[:].rearrange("p (k s) -> p k s", s=512),
                        a2a_out[h][:].rearrange("(k p) s -> p k s", p=128))
                    aout.append(ao[:].rearrange("p (k s) -> p k s", s=512))
                if DEBUG:
                    nc.sync.dma_start(dbg["a2ao0"].ap(), a2a_out[0][:])
                for np_ in range(2):
                    pso = {}
                    for mp in range(4):
                        for nn in range(2):
                            pso[(mp, nn)] = ops.tile(
                                [128, 512], F32, tag=f"o{mp}{nn}",
                                name=f"o{mp}{nn}")
                    for h in range(2):
                        for k8 in range(8):
                            kb = 2 * k8 + h
                            wo_t = opool.tile([128, 1024], BF16, tag="wo",
                                              bufs=4, name="wo_t")
                            nc.sync.dma_start(
                                wo_t[:],
                                woT.ap()[128 * kb:128 * (kb + 1),
                                         1024 * np_:1024 * (np_ + 1)])
                            for mp in range(4):
                                for nn in range(2):
                                    nc.tensor.matmul(
                                        pso[(mp, nn)][:],
                                        aout[h][:, k8,
                                                128 * mp:128 * (mp + 1)],
                                        wo_t[:, 512 * nn:512 * (nn + 1)],
                                        start=(h == 0 and k8 == 0),
                                        stop=(h == 1 and k8 == 7))
                    for mp in range(4):
                        for nn in range(2):
                            os_t = opool.tile([128, 512], F32, tag="osb",
                                              bufs=4, name="os_t")
                            nc.scalar.copy(os_t[:], pso[(mp, nn)][:])
                            nc.sync.dma_start(
                                out.ap()[128 * mp:128 * (mp + 1),
                                         1024 * np_ + 512 * nn:
                                         1024 * np_ + 512 * (nn + 1)],
                                os_t[:])

    nc.compile()
    return nc


_NC_CACHE = None


def _get_nc():
    global _NC_CACHE
    if _NC_CACHE is None:
        _NC_CACHE = _build()
    return _NC_CACHE


def _host_prep(inputs):
    hs = np.asarray(inputs["hidden_states"], dtype=np.float32)
    Wq = np.asarray(inputs["Wq"], dtype=np.float32)
    Wk = np.asarray(inputs["Wk"], dtype=np.float32)
    Wv = np.asarray(inputs["Wv"], dtype=np.float32)
    Wo = np.asarray(inputs["Wo"], dtype=np.float32)
    cqw = np.asarray(inputs["canon_q_w"], dtype=np.float32)
    ckw = np.asarray(inputs["canon_k_w"], dtype=np.float32)
    cvw = np.asarray(inputs["canon_v_w"], dtype=np.float32)
    qnw = np.asarray(inputs["q_norm_w"], dtype=np.float32)
    knw = np.asarray(inputs["k_norm_w"], dtype=np.float32)

    bf = ml_dtypes.bfloat16
    hsT = np.ascontiguousarray(
        np.concatenate([hs[0].T, hs[1].T], axis=1)).astype(bf)
    WqT, WkT, WvT = Wq.T, Wk.T, Wv.T
    woT = np.ascontiguousarray(Wo.T).astype(bf)

    inv_freq = 1.0 / (10000.0 ** (np.arange(0, DH, 2, dtype=np.float64) / DH))
    freqs = np.arange(S, dtype=np.float64)[:, None] * inv_freq
    emb = np.concatenate([freqs, freqs], axis=-1)
    cosT, sinT = np.cos(emb).T, np.sin(emb).T

    def make_rope(normw, scale):
        A = cosT * normw[:, None] * scale
        wswap = normw[(np.arange(DH) + 64) % DH]
        sign = np.where(np.arange(DH) < 64, -1.0, 1.0)
        Bc = sinT * wswap[:, None] * sign[:, None] * scale
        return (np.ascontiguousarray(A).astype(bf),
                np.ascontiguousarray(Bc).astype(bf))

    Aq, Bq = make_rope(qnw, SCALE)
    Ak, Bk = make_rope(knw, 1.0)

    p = np.arange(128)[:, None]
    f = np.arange(128)[None, :]
    maskd = np.where(p <= f, 0.0, NEG).astype(np.float32)

    in_maps = []
    for r in range(N_CORES):
        wTc = np.ascontiguousarray(np.concatenate(
            [WqT[:, 256 * r:256 * r + 256],
             WkT[:, 128 * r:128 * r + 128],
             WvT[:, 128 * r:128 * r + 128]], axis=1)).astype(bf)
        cwc = np.ascontiguousarray(np.concatenate(
            [cqw[256 * r:256 * r + 256],
             ckw[128 * r:128 * r + 128],
             cvw[128 * r:128 * r + 128]], axis=0)).astype(np.float32)
        in_maps.append({
            "hsT": hsT, "wT": wTc, "woT": woT, "cw": cwc,
            "ropeAq": Aq, "ropeBq": Bq, "ropeAk": Ak, "ropeBk": Bk,
            "maskd": maskd,
        })
    return in_maps


def kernel(**inputs):
    nc = _get_nc()
    in_maps = _host_prep(inputs)
    res = run_bass_kernel_spmd(nc, in_maps, core_ids=list(range(N_CORES)))
    full = np.empty((B, S, D), np.float32)
    for r in range(N_CORES):
        full[r // 4, 512 * (r % 4):512 * (r % 4 + 1), :] = res.results[r]["out"]
    return full


# revision 13
# speedup vs baseline: 1.0107x; 1.0085x over previous
"""Trainium2 Bass kernel for CanonCausalMultiheadAttn (v2).

Sharding: tensor-parallel over heads across 8 cores (2 q-heads + 1 kv-head
per core), both batches replicated. Each core computes its heads' attention
for both batches; two AllToAlls (one per local q-head) exchange attention
outputs so each core owns one (batch, seq-slice) of the final output
projection.

v2 structure (vs v1):
  - Phase A fuses QKV proj + canon conv + qk-rmsnorm + rope per 512-seq
    chunk so DVE/Pool/Scalar work overlaps the PE matmuls and the PE never
    idles (keeps the PE p-state at full clock).  V's canon runs on the Pool
    (gpsimd) engine; x^2 and rsqrt run on the Scalar engine (single act
    table: {Copy, Square, Rsqrt}).
  - Phase B attention keeps scores in [sk, q] layout but computes PV as
    out[dh, q] = sum_i va_i^T @ P_i (va as stationary operand), which lands
    directly in the AllToAll layout - no per-block DMA transposes and no
    [128,1] reciprocals.  Row sums come from ones-column matmuls; the
    1/rowsum is broadcast via a rank-1 matmul and applied with one DVE mul.
    A lag-2 software pipeline hides the exp (Scalar) latency.
  - The AllToAll is split per local q-head: cc_h0 fires halfway through
    attention; cc_h1 is hidden behind the even-channel half of the output
    projection (split by channel-block parity).
"""
import sys

sys.path.insert(0, '/opt/trn_rl_repo')

import numpy as np
import ml_dtypes

import concourse.bass as bass
import concourse.mybir as mybir
import concourse.tile as tile
from concourse import bacc
from concourse.bass_utils import run_bass_kernel_spmd

F32 = mybir.dt.float32
F32R = mybir.dt.float32r
F8 = mybir.dt.float8e4
PM_DR = mybir.MatmulPerfMode.DoubleRow
BF16 = mybir.dt.bfloat16
AF = mybir.ActivationFunctionType
ALU = mybir.AluOpType

B, S, D = 2, 2048, 2048
NH, NKV, DH = 16, 8, 128
K_CONV = 4
EPS = 1e-6
SCALE = 1.0 / float(np.sqrt(DH))
NEG = -1e9
N_CORES = 8
N_CHUNKS = S // 512     # 512-wide seq chunks per batch
FP8_QKV = False         # fp8e4 DoubleRow QKV: fast but fails the 2e-2 gate
N_SKB = S // 128        # 128-wide sk blocks per batch


def _build():
    nc = bacc.Bacc("TRN2", target_bir_lowering=False, debug=False,
                   num_devices=N_CORES)

    QKV_DT = F8 if FP8_QKV else BF16
    hsT = nc.dram_tensor("hsT", [D, B * S], QKV_DT, kind="ExternalInput")
    wT = nc.dram_tensor("wT", [D, 512], QKV_DT, kind="ExternalInput")
    woT = nc.dram_tensor("woT", [D, D], BF16, kind="ExternalInput")
    cw = nc.dram_tensor("cw", [512, K_CONV], F32, kind="ExternalInput")
    ropeAq = nc.dram_tensor("ropeAq", [DH, S], BF16, kind="ExternalInput")
    ropeBq = nc.dram_tensor("ropeBq", [DH, S], BF16, kind="ExternalInput")
    ropeAk = nc.dram_tensor("ropeAk", [DH, S], BF16, kind="ExternalInput")
    ropeBk = nc.dram_tensor("ropeBk", [DH, S], BF16, kind="ExternalInput")
    maskd = nc.dram_tensor("maskd", [128, 128], F32, kind="ExternalInput")
    out = nc.dram_tensor("out", [512, D], F32, kind="ExternalOutput")
    import os
    DEBUG = os.environ.get("KDBG", "") == "1"
    dbg = {}
    if DEBUG:
        dbg["roped00"] = nc.dram_tensor("d_roped00", [128, S], BF16, kind="ExternalOutput")
        dbg["roped02"] = nc.dram_tensor("d_roped02", [128, S], BF16, kind="ExternalOutput")
        dbg["vaug0"] = nc.dram_tensor("d_vaug0", [128, N_SKB * 128], BF16, kind="ExternalOutput")
        dbg["rkt0"] = nc.dram_tensor("d_rkt0", [128, N_SKB], F32, kind="ExternalOutput")
        dbg["pt00"] = nc.dram_tensor("d_pt00", [128, 512], BF16, kind="ExternalOutput")
        dbg["rs00"] = nc.dram_tensor("d_rs00", [1, 512], F32, kind="ExternalOutput")
        dbg["oa00"] = nc.dram_tensor("d_oa00", [128, 512], BF16, kind="ExternalOutput")
        dbg["a2ao0"] = nc.dram_tensor("d_a2ao0", [N_CORES * 128, 512], BF16, kind="ExternalOutput")

    with tile.TileContext(nc) as tc:
        with tc.tile_pool(name="const", bufs=1) as cpool, \
             tc.tile_pool(name="persist", bufs=1) as pers, \
             tc.tile_pool(name="dram", bufs=1, space="DRAM") as dram:

            # ---- constants (weights first so QKV can start ASAP) ----
            wvq = []
            for q4 in range(4):
                t = cpool.tile([128, 4 * 512], QKV_DT, tag=f"wsb{q4}",
                               name=f"wsb{q4}")
                nc.sync.dma_start(
                    t[:].rearrange("p (k s) -> p k s", s=512),
                    wT.ap()[512 * q4:512 * (q4 + 1), :]
                    .rearrange("(k p) s -> p k s", p=128))
                wvq.append(t[:].rearrange("p (k s) -> p k s", s=512))
            mask_sb = cpool.tile([128, 128], F32, tag="mask")
            nc.sync.dma_start(mask_sb[:], maskd.ap())
            ropes = {}
            rope_dram = {"Aq": ropeAq, "Bq": ropeBq, "Ak": ropeAk,
                         "Bk": ropeBk}
            for nm in rope_dram:
                ropes[nm] = cpool.tile([DH, S], BF16, tag=f"rope{nm}",
                                       name=f"rope{nm}")

            def load_ropes():
                for nm, t in rope_dram.items():
                    nc.sync.dma_start(ropes[nm][:], t.ap())
            cw_sb = []
            for mt in range(4):
                t = cpool.tile([128, K_CONV], F32, tag=f"cw{mt}", name=f"cw{mt}")
                nc.sync.dma_start(t[:], cw.ap()[128 * mt:128 * mt + 128, :])
                cw_sb.append(t)
            ones_col = cpool.tile([128, 1], BF16, tag="oc")
            nc.vector.memset(ones_col[:], 1.0)
            eps_sb = cpool.tile([1, 1], F32, tag="eps")
            nc.vector.memset(eps_sb[:], EPS)
            ones_row_f = cpool.tile([1, 128], F32, tag="orf")
            nc.vector.memset(ones_row_f[:], 1.0)
            ones_row = cpool.tile([1, 128], F32R, tag="or")
            nc.scalar.copy(ones_row[:], ones_row_f[:])
            s0_sb = []
            for mt in range(4):
                t = cpool.tile([128, 1], F32, tag=f"s0{mt}", name=f"s0{mt}")
                nc.vector.tensor_scalar_add(t[:], cw_sb[mt][:, 0:1], 1.0)
                s0_sb.append(t)

            # persistent per-(b,mt) tiles
            roped = {}   # (b, mt<3) -> [128, S] bf16 (q0,q1 scaled by rstd)
            vaug = {}    # b -> [128, N_SKB*128] bf16 (transposed V)
            rstdkT = {}  # b -> [128, N_SKB] f32 (k rstd, transposed)

            for b in range(B):
                vaug[b] = pers.tile([128, N_SKB * 128], BF16, tag=f"vaug{b}",
                                    name=f"vaug{b}")
                rstdkT[b] = pers.tile([128, N_SKB], F32, tag=f"rstdkT{b}",
                                      name=f"rstdkT{b}")
                for mt in range(3):
                    roped[(b, mt)] = pers.tile([128, S], BF16,
                                               tag=f"roped{b}{mt}",
                                               name=f"roped{b}{mt}")

            # ============ phase A: QKV + canon + norm + rope ============
            # Two-stage pipeline: stage1(n) = hs DMA + QKV matmuls + psum->bf16
            # copies; stage2(n) = canon + norm + rope.  stage2(n-1) is emitted
            # after stage1(n) so its colsum/bcast matmuls never stall the PE.
            with tc.tile_pool(name="awork", bufs=1) as bw, \
                 tc.tile_pool(name="qps", bufs=1, space="PSUM") as qps, \
                 tc.tile_pool(name="nps", bufs=2, space="PSUM") as nps, \
                 tc.tile_pool(name="bps", bufs=1, space="PSUM") as bps:
                rk_ds = {b: dram.tile([N_SKB, 128], F32, tag=f"rkd{b}",
                                      name=f"rk_d{b}") for b in range(B)}
                if True:

                    def stage1(n, b):
                        rk_d = rk_ds[b]
                        lo = 512 * n
                        hsp = []
                        for q4 in range(4):
                            t = bw.tile([128, 4 * 512], QKV_DT,
                                        tag=f"hs{q4}", bufs=2,
                                        name=f"hs{q4}")
                            nc.sync.dma_start(
                                t[:].rearrange("p (k s) -> p k s", s=512),
                                hsT.ap()[512 * q4:512 * (q4 + 1),
                                         b * S + lo:b * S + lo + 512]
                                .rearrange("(k p) s -> p k s", p=128))
                            hsp.append(t[:].rearrange("p (k s) -> p k s",
                                                      s=512))
                        psums = [qps.tile([128, 512], F32, tag=f"qk{mt}",
                                          name=f"qk{mt}") for mt in range(4)]
                        if FP8_QKV:
                            for k8 in range(8):
                                k = 2 * k8
                                for mt in range(4):
                                    nc.tensor.matmul(
                                        psums[mt][:],
                                        wvq[k // 4][:, k % 4:k % 4 + 2,
                                                    128 * mt:128 * (mt + 1)],
                                        hsp[k // 4][:, k % 4:k % 4 + 2, :],
                                        start=(k8 == 0), stop=(k8 == 7),
                                        perf_mode=PM_DR)
                        else:
                            for k in range(16):
                                for mt in range(4):
                                    nc.tensor.matmul(
                                        psums[mt][:],
                                        wvq[k // 4][:, k % 4,
                                                    128 * mt:128 * (mt + 1)],
                                        hsp[k // 4][:, k % 4, :],
                                        start=(k == 0), stop=(k == 15))
                        raws = []
                        for mt in range(4):
                            raw_c = bw.tile([128, 512], BF16, tag=f"rawc{mt}",
                                            bufs=3, name=f"rawc{mt}")
                            nc.scalar.copy(raw_c[:], psums[mt][:])
                            raws.append(raw_c)
                        return raws

                    def stage2(n, raws, prev, b):
                        rk_d = rk_ds[b]
                        lo = 512 * n
                        cn = {}
                        for mt in range(4):
                            c = bw.tile([128, 512], BF16, tag=f"cn{mt}",
                                        bufs=2, name=f"cn{mt}")
                            nc.vector.tensor_scalar_mul(c[:], raws[mt][:],
                                                        s0_sb[mt][:])
                            for k in range(1, K_CONV):
                                nc.vector.scalar_tensor_tensor(
                                    c[:, k:512], raws[mt][:, 0:512 - k],
                                    cw_sb[mt][:, k:k + 1], c[:, k:512],
                                    ALU.mult, ALU.add)
                                if prev is not None:
                                    nc.vector.scalar_tensor_tensor(
                                        c[:, 0:k],
                                        prev[mt][:, 512 - k:512],
                                        cw_sb[mt][:, k:k + 1], c[:, 0:k],
                                        ALU.mult, ALU.add)
                            cn[mt] = c
                        for i in range(4):
                            nc.sync.dma_start_transpose(
                                vaug[b][:, 128 * (4 * n + i):
                                        128 * (4 * n + i + 1)],
                                cn[3][:, 128 * i:128 * (i + 1)])
                        # rmsnorm rstd on the scalar engine:
                        # rstd = exp(-0.5*ln(meansq + eps))
                        rstd = {}
                        for mt in range(3):
                            sq = bw.tile([128, 512], BF16, tag="sq", bufs=2,
                                         name="sq")
                            nc.scalar.activation(sq[:], cn[mt][:], AF.Square)
                            sp = nps.tile([1, 512], F32, tag="ssq")
                            nc.tensor.matmul(sp[:], ones_col[:], sq[:],
                                             start=True, stop=True)
                            rs = bw.tile([1, 512], F32R if mt < 2 else F32,
                                         tag=f"rstd{mt}", bufs=2,
                                         name=f"rstd{mt}")
                            with nc.allow_low_precision(
                                    reason="rstd f32r is plenty"):
                                nc.scalar.activation(rs[:], sp[:],
                                                     AF.Abs_reciprocal_sqrt,
                                                     bias=eps_sb[:],
                                                     scale=1.0 / DH)
                            rstd[mt] = rs
                        nc.sync.dma_start(rk_d[4 * n:4 * (n + 1), :],
                                          rstd[2][:])
                        for mt in range(3):
                            is_q = mt < 2
                            A_ = ropes["Aq"] if is_q else ropes["Ak"]
                            B_ = ropes["Bq"] if is_q else ropes["Bk"]
                            c = cn[mt]
                            sh = bw.tile([128, 512], BF16, tag="sh", bufs=2,
                                         name="sh")
                            nc.sync.dma_start(sh[0:64, :], c[64:128, :])
                            nc.sync.dma_start(sh[64:128, :], c[0:64, :])
                            t1 = bw.tile([128, 512], BF16, tag="t1", bufs=2,
                                         name="t1")
                            nc.vector.tensor_mul(t1[:], sh[:],
                                                 B_[:, lo:lo + 512])
                            t2 = bw.tile([128, 512], BF16, tag="t2", bufs=2,
                                         name="t2")
                            nc.vector.tensor_mul(t2[:], c[:],
                                                 A_[:, lo:lo + 512])
                            ro = roped[(b, mt)]
                            if is_q:
                                bc = bps.tile([128, 512], F32, tag=f"bc{mt}",
                                              name=f"bc{mt}")
                                nc.tensor.matmul(bc[:], ones_row[:],
                                                 rstd[mt][:], start=True,
                                                 stop=True)
                                bcb = bw.tile([128, 512], BF16,
                                              tag="bcb", bufs=2, name="bcb")
                                nc.scalar.copy(bcb[:], bc[:])
                                t3 = bw.tile([128, 512], BF16, tag="t3",
                                             bufs=2, name="t3")
                                nc.vector.tensor_add(t3[:], t1[:], t2[:])
                                nc.vector.tensor_mul(ro[:, lo:lo + 512],
                                                     t3[:], bcb[:])
                            else:
                                nc.vector.tensor_add(ro[:, lo:lo + 512],
                                                     t1[:], t2[:])

                    def finish_batch(b):
                        nc.sync.dma_start(rstdkT[b][:],
                                          rk_ds[b][:].rearrange("i p -> p i"))
                        if DEBUG and b == 0:
                            nc.sync.dma_start(dbg["roped00"].ap(),
                                              roped[(0, 0)][:])
                            nc.sync.dma_start(dbg["roped02"].ap(),
                                              roped[(0, 2)][:])
                            nc.sync.dma_start(dbg["vaug0"].ap(), vaug[0][:])
                            nc.sync.dma_start(dbg["rkt0"].ap(), rstdkT[0][:])

                    pairs = [(b, n) for b in range(B) for n in range(N_CHUNKS)]
                    prev_by_b = {0: None, 1: None}
                    pend_st2 = None  # (b, n, raws)
                    for (b, n) in pairs:
                        raws = stage1(n, b)
                        if (b, n) == (0, 0):
                            load_ropes()
                        if pend_st2 is not None:
                            pb, pn, praws = pend_st2
                            stage2(pn, praws, prev_by_b[pb], pb)
                            prev_by_b[pb] = praws
                            if pn == N_CHUNKS - 1:
                                finish_batch(pb)
                        pend_st2 = (b, n, raws)
                    pb, pn, praws = pend_st2
                    stage2(pn, praws, prev_by_b[pb], pb)
                    finish_batch(pb)

            # ======================= attention =======================
            # a2a buffers: one per local q-head, [8*128 rows, 512 cols]
            a2a_in = [dram.tile([N_CORES * 128, 512], BF16, tag=f"a2ai{h}",
                                name=f"a2a_in{h}") for h in range(2)]
            a2a_out = [dram.tile([N_CORES * 128, 512], BF16, tag=f"a2ao{h}",
                                 name=f"a2a_out{h}") for h in range(2)]

            with tc.tile_pool(name="scps", bufs=4, space="PSUM") as scps, \
                 tc.tile_pool(name="pvps", bufs=2, space="PSUM") as pvps, \
                 tc.tile_pool(name="rsps", bufs=1, space="PSUM") as rsps, \
                 tc.tile_pool(name="abps", bufs=1, space="PSUM") as abps, \
                 tc.tile_pool(name="apool", bufs=1) as apool:
                for h in range(2):
                    for b in range(B):
                        KT = roped[(b, 2)]
                        QT = roped[(b, h)]
                        va = vaug[b]
                        rkt = rstdkT[b]
                        for j in range(N_CHUNKS):
                            nij = 4 * j + 4
                            out_ps = pvps.tile([128, 512], F32, tag="pv",
                                               name="out_ps")
                            rs_ps = rsps.tile([1, 512], F32, tag="rs",
                                              name="rs_ps")

                            def emit_pv(i, pt):
                                nc.tensor.matmul(
                                    out_ps[:],
                                    va[:, 128 * i:128 * (i + 1)], pt[:],
                                    start=(i == 0), stop=(i == nij - 1))
                                nc.tensor.matmul(
                                    rs_ps[:], ones_col[:], pt[:],
                                    start=(i == 0), stop=(i == nij - 1))

                            pend = []
                            for i in range(nij):
                                r = i - 4 * j
                                off = 128 * max(r, 0)
                                sc = scps.tile([128, 512], F32, tag="sc",
                                               name="sc")
                                nc.tensor.matmul(
                                    sc[:, off:512],
                                    KT[:, 128 * i:128 * (i + 1)],
                                    QT[:, 512 * j + off:512 * (j + 1)],
                                    start=True, stop=True)
                                if r >= 0:
                                    nc.vector.tensor_add(
                                        sc[:, off:off + 128],
                                        sc[:, off:off + 128], mask_sb[:])
                                pt = apool.tile([128, 512], BF16, tag="pt",
                                                bufs=5, name="pt")
                                if off > 0:
                                    nc.vector.memset(pt[:, 0:off], 0.0)
                                nc.scalar.activation(
                                    pt[:, off:512], sc[:, off:512], AF.Exp,
                                    scale=rkt[:, i:i + 1])
                                if DEBUG and h == 0 and b == 0 and j == 0 and i == 0:
                                    nc.sync.dma_start(dbg["pt00"].ap(), pt[:])
                                pend.append((i, pt))
                                if len(pend) > 3:
                                    emit_pv(*pend.pop(0))
                            for it in pend:
                                emit_pv(*it)
                            # normalize and ship
                            rsb = apool.tile([1, 512], F32, tag="rsb",
                                             bufs=2, name="rsb")
                            nc.vector.tensor_copy(rsb[:], rs_ps[:])
                            rr = apool.tile([1, 512], F32R, tag="rr", bufs=2,
                                            name="rr")
                            with nc.allow_low_precision(
                                    reason="softmax denom f32r ample"):
                                nc.vector.reciprocal(rr[:], rsb[:])
                            bcn = abps.tile([128, 512], F32, tag="bcn",
                                            name="bcn")
                            nc.tensor.matmul(bcn[:], ones_row[:], rr[:],
                                             start=True, stop=True)
                            bcs = apool.tile([128, 512], F32, tag="bcs",
                                             bufs=2, name="bcs")
                            nc.vector.tensor_copy(bcs[:], bcn[:])
                            oa = apool.tile([128, 512], BF16, tag="oa",
                                            bufs=2, name="oa")
                            nc.vector.tensor_mul(oa[:], out_ps[:], bcs[:])
                            rd = 4 * b + j
                            if DEBUG and h == 0 and b == 0 and j == 0:
                                nc.sync.dma_start(dbg["oa00"].ap(), oa[:])
                            nc.sync.dma_start(
                                a2a_in[h][128 * rd:128 * (rd + 1), :], oa[:])
                    # per-head AllToAll as soon as this head's blocks done
                    nc.gpsimd.collective_compute(
                        "AllToAll", ALU.bypass,
                        replica_groups=[list(range(N_CORES))],
                        ins=[a2a_in[h].opt()], outs=[a2a_out[h].opt()],
                        cc_dim="Partition")

            # ====================== out projection ====================
            # channel block 2*s+h of the gathered activation lives in
            # a2a_out[h] src-block s.  Split the contraction by h so the
            # h=1 AllToAll hides behind the h=0 half of the matmul.
            with tc.tile_pool(name="opool", bufs=1) as opool, \
                 tc.tile_pool(name="ops", bufs=1, space="PSUM") as ops:
                aout = []
                for h in range(2):
                    ao = opool.tile([128, 8 * 512], BF16, tag=f"ao{h}",
                                    name=f"ao{h}")
                    nc.sync.dma_start(
                        ao[:].rearrange("p (k s) -> p k s", s=512),
                        a2a_out[h][:].rearrange("(k p) s -> p k s", p=128))
                    aout.append(ao[:].rearrange("p (k s) -> p k s", s=512))
                if DEBUG:
                    nc.sync.dma_start(dbg["a2ao0"].ap(), a2a_out[0][:])
                for np_ in range(2):
                    pso = {}
                    for mp in range(4):
                        for nn in range(2):
                            pso[(mp, nn)] = ops.tile(
                                [128, 512], F32, tag=f"o{mp}{nn}",
                                name=f"o{mp}{nn}")
                    for h in range(2):
                        for k8 in range(8):
                            kb = 2 * k8 + h
                            wo_t = opool.tile([128, 1024], BF16, tag="wo",
                                              bufs=4, name="wo_t")
                            nc.sync.dma_start(
                                wo_t[:],
                                woT.ap()[128 * kb:128 * (kb + 1),
                                         1024 * np_:1024 * (np_ + 1)])
                            for mp in range(4):
                                for nn in range(2):
                                    nc.tensor.matmul(
                                        pso[(mp, nn)][:],
                                        aout[h][:, k8,
                                                128 * mp:128 * (mp + 1)],
                                        wo_t[:, 512 * nn:512 * (nn + 1)],
                                        start=(h == 0 and k8 == 0),
                                        stop=(h == 1 and k8 == 7))
                    for mp in range(4):
                        for nn in range(2):
                            os_t = opool.tile([128, 512], F32, tag="osb",
                                              bufs=4, name="os_t")
                            nc.scalar.copy(os_t[:], pso[(mp, nn)][:])
                            nc.sync.dma_start(
                                out.ap()[128 * mp:128 * (mp + 1),
                                         1024 * np_ + 512 * nn:
                                         1024 * np_ + 512 * (nn + 1)],
                                os_t[:])

    nc.compile()
    return nc


_NC_CACHE = None


def _get_nc():
    global _NC_CACHE
    if _NC_CACHE is None:
        _NC_CACHE = _build()
    return _NC_CACHE


def _host_prep(inputs):
    hs = np.asarray(inputs["hidden_states"], dtype=np.float32)
    Wq = np.asarray(inputs["Wq"], dtype=np.float32)
    Wk = np.asarray(inputs["Wk"], dtype=np.float32)
    Wv = np.asarray(inputs["Wv"], dtype=np.float32)
    Wo = np.asarray(inputs["Wo"], dtype=np.float32)
    cqw = np.asarray(inputs["canon_q_w"], dtype=np.float32)
    ckw = np.asarray(inputs["canon_k_w"], dtype=np.float32)
    cvw = np.asarray(inputs["canon_v_w"], dtype=np.float32)
    qnw = np.asarray(inputs["q_norm_w"], dtype=np.float32)
    knw = np.asarray(inputs["k_norm_w"], dtype=np.float32)

    bf = ml_dtypes.bfloat16
    qkv_dt = ml_dtypes.float8_e4m3fn if FP8_QKV else bf
    hsT = np.ascontiguousarray(
        np.concatenate([hs[0].T, hs[1].T], axis=1)).astype(qkv_dt)
    WqT, WkT, WvT = Wq.T, Wk.T, Wv.T
    woT = np.ascontiguousarray(Wo.T).astype(bf)

    inv_freq = 1.0 / (10000.0 ** (np.arange(0, DH, 2, dtype=np.float64) / DH))
    freqs = np.arange(S, dtype=np.float64)[:, None] * inv_freq
    emb = np.concatenate([freqs, freqs], axis=-1)
    cosT, sinT = np.cos(emb).T, np.sin(emb).T

    def make_rope(normw, scale):
        A = cosT * normw[:, None] * scale
        wswap = normw[(np.arange(DH) + 64) % DH]
        sign = np.where(np.arange(DH) < 64, -1.0, 1.0)
        Bc = sinT * wswap[:, None] * sign[:, None] * scale
        return (np.ascontiguousarray(A).astype(bf),
                np.ascontiguousarray(Bc).astype(bf))

    Aq, Bq = make_rope(qnw, SCALE)
    Ak, Bk = make_rope(knw, 1.0)

    p = np.arange(128)[:, None]
    f = np.arange(128)[None, :]
    maskd = np.where(p <= f, 0.0, NEG).astype(np.float32)

    in_maps = []
    for r in range(N_CORES):
        wTc = np.ascontiguousarray(np.concatenate(
            [WqT[:, 256 * r:256 * r + 256],
             WkT[:, 128 * r:128 * r + 128],
             WvT[:, 128 * r:128 * r + 128]], axis=1)).astype(qkv_dt)
        cwc = np.ascontiguousarray(np.concatenate(
            [cqw[256 * r:256 * r + 256],
             ckw[128 * r:128 * r + 128],
             cvw[128 * r:128 * r + 128]], axis=0)).astype(np.float32)
        in_maps.append({
            "hsT": hsT, "wT": wTc, "woT": woT, "cw": cwc,
            "ropeAq": Aq, "ropeBq": Bq, "ropeAk": Ak, "ropeBk": Bk,
            "maskd": maskd,
        })
    return in_maps


def kernel(**inputs):
    nc = _get_nc()
    in_maps = _host_prep(inputs)
    res = run_bass_kernel_spmd(nc, in_maps, core_ids=list(range(N_CORES)))
    full = np.empty((B, S, D), np.float32)
    for r in range(N_CORES):
        full[r // 4, 512 * (r % 4):512 * (r % 4 + 1), :] = res.results[r]["out"]
    return full


# revision 14
# speedup vs baseline: 1.0379x; 1.0269x over previous
"""Trainium2 Bass kernel for CanonCausalMultiheadAttn (v2).

Sharding: tensor-parallel over heads across 8 cores (2 q-heads + 1 kv-head
per core), both batches replicated. Each core computes its heads' attention
for both batches; two AllToAlls (one per local q-head) exchange attention
outputs so each core owns one (batch, seq-slice) of the final output
projection.

v2 structure (vs v1):
  - Phase A fuses QKV proj + canon conv + qk-rmsnorm + rope per 512-seq
    chunk so DVE/Pool/Scalar work overlaps the PE matmuls and the PE never
    idles (keeps the PE p-state at full clock).  V's canon runs on the Pool
    (gpsimd) engine; x^2 and rsqrt run on the Scalar engine (single act
    table: {Copy, Square, Rsqrt}).
  - Phase B attention keeps scores in [sk, q] layout but computes PV as
    out[dh, q] = sum_i va_i^T @ P_i (va as stationary operand), which lands
    directly in the AllToAll layout - no per-block DMA transposes and no
    [128,1] reciprocals.  Row sums come from ones-column matmuls; the
    1/rowsum is broadcast via a rank-1 matmul and applied with one DVE mul.
    A lag-2 software pipeline hides the exp (Scalar) latency.
  - The AllToAll is split per local q-head: cc_h0 fires halfway through
    attention; cc_h1 is hidden behind the even-channel half of the output
    projection (split by channel-block parity).
"""
import sys

sys.path.insert(0, '/opt/trn_rl_repo')

import numpy as np
import ml_dtypes

import concourse.bass as bass
import concourse.mybir as mybir
import concourse.tile as tile
from concourse import bacc
from concourse.bass_utils import run_bass_kernel_spmd

F32 = mybir.dt.float32
F32R = mybir.dt.float32r
F8 = mybir.dt.float8e4
PM_DR = mybir.MatmulPerfMode.DoubleRow
BF16 = mybir.dt.bfloat16
AF = mybir.ActivationFunctionType
ALU = mybir.AluOpType

B, S, D = 2, 2048, 2048
NH, NKV, DH = 16, 8, 128
K_CONV = 4
EPS = 1e-6
SCALE = 1.0 / float(np.sqrt(DH))
NEG = -1e9
N_CORES = 8
N_CHUNKS = S // 512     # 512-wide seq chunks per batch
FP8_QKV = False         # fp8e4 DoubleRow QKV: fast but fails the 2e-2 gate
N_SKB = S // 128        # 128-wide sk blocks per batch


def _build():
    nc = bacc.Bacc("TRN2", target_bir_lowering=False, debug=False,
                   num_devices=N_CORES)

    QKV_DT = F8 if FP8_QKV else BF16
    hsT = nc.dram_tensor("hsT", [D, B * S], QKV_DT, kind="ExternalInput")
    wT = nc.dram_tensor("wT", [D, 512], QKV_DT, kind="ExternalInput")
    woT = nc.dram_tensor("woT", [D, D], BF16, kind="ExternalInput")
    cw = nc.dram_tensor("cw", [512, K_CONV], F32, kind="ExternalInput")
    ropeAq = nc.dram_tensor("ropeAq", [DH, S], BF16, kind="ExternalInput")
    ropeBq = nc.dram_tensor("ropeBq", [DH, S], BF16, kind="ExternalInput")
    ropeAk = nc.dram_tensor("ropeAk", [DH, S], BF16, kind="ExternalInput")
    ropeBk = nc.dram_tensor("ropeBk", [DH, S], BF16, kind="ExternalInput")
    maskd = nc.dram_tensor("maskd", [128, 128], F32, kind="ExternalInput")
    out = nc.dram_tensor("out", [512, D], F32, kind="ExternalOutput")
    import os
    DEBUG = os.environ.get("KDBG", "") == "1"
    dbg = {}
    if DEBUG:
        dbg["roped00"] = nc.dram_tensor("d_roped00", [128, S], BF16, kind="ExternalOutput")
        dbg["roped02"] = nc.dram_tensor("d_roped02", [128, S], BF16, kind="ExternalOutput")
        dbg["vaug0"] = nc.dram_tensor("d_vaug0", [128, N_SKB * 128], BF16, kind="ExternalOutput")
        dbg["rkt0"] = nc.dram_tensor("d_rkt0", [128, N_SKB], F32, kind="ExternalOutput")
        dbg["pt00"] = nc.dram_tensor("d_pt00", [128, 512], BF16, kind="ExternalOutput")
        dbg["rs00"] = nc.dram_tensor("d_rs00", [1, 512], F32, kind="ExternalOutput")
        dbg["oa00"] = nc.dram_tensor("d_oa00", [128, 512], BF16, kind="ExternalOutput")
        dbg["a2ao0"] = nc.dram_tensor("d_a2ao0", [N_CORES * 128, 512], BF16, kind="ExternalOutput")

    with tile.TileContext(nc) as tc:
        with tc.tile_pool(name="const", bufs=1) as cpool, \
             tc.tile_pool(name="persist", bufs=1) as pers, \
             tc.tile_pool(name="dram", bufs=1, space="DRAM") as dram:

            # ---- constants (weights first so QKV can start ASAP) ----
            wvq = []
            for q4 in range(4):
                t = cpool.tile([128, 4 * 512], QKV_DT, tag=f"wsb{q4}",
                               name=f"wsb{q4}")
                nc.sync.dma_start(
                    t[:].rearrange("p (k s) -> p k s", s=512),
                    wT.ap()[512 * q4:512 * (q4 + 1), :]
                    .rearrange("(k p) s -> p k s", p=128))
                wvq.append(t[:].rearrange("p (k s) -> p k s", s=512))
            mask_sb = cpool.tile([128, 128], F32, tag="mask")
            nc.sync.dma_start(mask_sb[:], maskd.ap())
            ropes = {}
            rope_dram = {"Aq": ropeAq, "Bq": ropeBq, "Ak": ropeAk,
                         "Bk": ropeBk}
            for nm in rope_dram:
                ropes[nm] = cpool.tile([DH, S], BF16, tag=f"rope{nm}",
                                       name=f"rope{nm}")

            def load_ropes():
                for nm, t in rope_dram.items():
                    nc.sync.dma_start(ropes[nm][:], t.ap())
            cw_sb = []
            for mt in range(4):
                t = cpool.tile([128, K_CONV], F32, tag=f"cw{mt}", name=f"cw{mt}")
                nc.sync.dma_start(t[:], cw.ap()[128 * mt:128 * mt + 128, :])
                cw_sb.append(t)
            ones_col = cpool.tile([128, 1], BF16, tag="oc")
            nc.vector.memset(ones_col[:], 1.0)
            eps_sb = cpool.tile([1, 1], F32, tag="eps")
            nc.vector.memset(eps_sb[:], EPS)
            ones_row_f = cpool.tile([1, 128], F32, tag="orf")
            nc.vector.memset(ones_row_f[:], 1.0)
            ones_row = cpool.tile([1, 128], F32R, tag="or")
            nc.scalar.copy(ones_row[:], ones_row_f[:])
            s0_sb = []
            for mt in range(4):
                t = cpool.tile([128, 1], F32, tag=f"s0{mt}", name=f"s0{mt}")
                nc.vector.tensor_scalar_add(t[:], cw_sb[mt][:, 0:1], 1.0)
                s0_sb.append(t)

            # persistent per-(b,mt) tiles
            roped = {}   # (b, mt<3) -> [128, S] bf16 (q0,q1 scaled by rstd)
            vaug = {}    # b -> [128, N_SKB*128] bf16 (transposed V)
            rstdkT = {}  # b -> [128, N_SKB] f32 (k rstd, transposed)

            for b in range(B):
                vaug[b] = pers.tile([128, N_SKB * 128], BF16, tag=f"vaug{b}",
                                    name=f"vaug{b}")
                rstdkT[b] = pers.tile([128, N_SKB], F32, tag=f"rstdkT{b}",
                                      name=f"rstdkT{b}")
                for mt in range(3):
                    roped[(b, mt)] = pers.tile([128, S], BF16,
                                               tag=f"roped{b}{mt}",
                                               name=f"roped{b}{mt}")

            # ============ phase A: QKV + canon + norm + rope ============
            # Two-stage pipeline: stage1(n) = hs DMA + QKV matmuls + psum->bf16
            # copies; stage2(n) = canon + norm + rope.  stage2(n-1) is emitted
            # after stage1(n) so its colsum/bcast matmuls never stall the PE.
            with tc.tile_pool(name="awork", bufs=1) as bw, \
                 tc.tile_pool(name="qps", bufs=1, space="PSUM") as qps, \
                 tc.tile_pool(name="nps", bufs=2, space="PSUM") as nps, \
                 tc.tile_pool(name="bps", bufs=1, space="PSUM") as bps:
                rk_ds = {b: dram.tile([N_SKB, 128], F32, tag=f"rkd{b}",
                                      name=f"rk_d{b}") for b in range(B)}
                if True:

                    def stage1(n, b):
                        rk_d = rk_ds[b]
                        lo = 512 * n
                        hsp = []
                        for q4 in range(4):
                            t = bw.tile([128, 4 * 512], QKV_DT,
                                        tag=f"hs{q4}", bufs=2,
                                        name=f"hs{q4}")
                            nc.sync.dma_start(
                                t[:].rearrange("p (k s) -> p k s", s=512),
                                hsT.ap()[512 * q4:512 * (q4 + 1),
                                         b * S + lo:b * S + lo + 512]
                                .rearrange("(k p) s -> p k s", p=128))
                            hsp.append(t[:].rearrange("p (k s) -> p k s",
                                                      s=512))
                        psums = [qps.tile([128, 512], F32, tag=f"qk{mt}",
                                          name=f"qk{mt}") for mt in range(4)]
                        if FP8_QKV:
                            for k8 in range(8):
                                k = 2 * k8
                                for mt in range(4):
                                    nc.tensor.matmul(
                                        psums[mt][:],
                                        wvq[k // 4][:, k % 4:k % 4 + 2,
                                                    128 * mt:128 * (mt + 1)],
                                        hsp[k // 4][:, k % 4:k % 4 + 2, :],
                                        start=(k8 == 0), stop=(k8 == 7),
                                        perf_mode=PM_DR)
                        else:
                            for k in range(16):
                                for mt in range(4):
                                    nc.tensor.matmul(
                                        psums[mt][:],
                                        wvq[k // 4][:, k % 4,
                                                    128 * mt:128 * (mt + 1)],
                                        hsp[k // 4][:, k % 4, :],
                                        start=(k == 0), stop=(k == 15))
                        raws = []
                        for mt in range(4):
                            raw_c = bw.tile([128, 512], BF16, tag=f"rawc{mt}",
                                            bufs=3, name=f"rawc{mt}")
                            nc.scalar.copy(raw_c[:], psums[mt][:])
                            raws.append(raw_c)
                        return raws

                    def stage2(n, raws, prev, b):
                        rk_d = rk_ds[b]
                        lo = 512 * n
                        cn = {}
                        for mt in range(4):
                            c = bw.tile([128, 512], BF16, tag=f"cn{mt}",
                                        bufs=2, name=f"cn{mt}")
                            nc.vector.tensor_scalar_mul(c[:], raws[mt][:],
                                                        s0_sb[mt][:])
                            for k in range(1, K_CONV):
                                nc.vector.scalar_tensor_tensor(
                                    c[:, k:512], raws[mt][:, 0:512 - k],
                                    cw_sb[mt][:, k:k + 1], c[:, k:512],
                                    ALU.mult, ALU.add)
                                if prev is not None:
                                    nc.vector.scalar_tensor_tensor(
                                        c[:, 0:k],
                                        prev[mt][:, 512 - k:512],
                                        cw_sb[mt][:, k:k + 1], c[:, 0:k],
                                        ALU.mult, ALU.add)
                            cn[mt] = c
                        for i in range(4):
                            nc.sync.dma_start_transpose(
                                vaug[b][:, 128 * (4 * n + i):
                                        128 * (4 * n + i + 1)],
                                cn[3][:, 128 * i:128 * (i + 1)])
                        # rmsnorm rstd on the scalar engine:
                        # rstd = exp(-0.5*ln(meansq + eps))
                        rstd = {}
                        for mt in range(3):
                            sq = bw.tile([128, 512], BF16, tag="sq", bufs=2,
                                         name="sq")
                            nc.scalar.activation(sq[:], cn[mt][:], AF.Square)
                            sp = nps.tile([1, 512], F32, tag="ssq")
                            nc.tensor.matmul(sp[:], ones_col[:], sq[:],
                                             start=True, stop=True)
                            rs = bw.tile([1, 512], F32R if mt < 2 else F32,
                                         tag=f"rstd{mt}", bufs=2,
                                         name=f"rstd{mt}")
                            with nc.allow_low_precision(
                                    reason="rstd f32r is plenty"):
                                nc.scalar.activation(rs[:], sp[:],
                                                     AF.Abs_reciprocal_sqrt,
                                                     bias=eps_sb[:],
                                                     scale=1.0 / DH)
                            rstd[mt] = rs
                        nc.sync.dma_start(rk_d[4 * n:4 * (n + 1), :],
                                          rstd[2][:])
                        for mt in range(3):
                            is_q = mt < 2
                            A_ = ropes["Aq"] if is_q else ropes["Ak"]
                            B_ = ropes["Bq"] if is_q else ropes["Bk"]
                            c = cn[mt]
                            sh = bw.tile([128, 512], BF16, tag="sh", bufs=2,
                                         name="sh")
                            nc.sync.dma_start(sh[0:64, :], c[64:128, :])
                            nc.sync.dma_start(sh[64:128, :], c[0:64, :])
                            t1 = bw.tile([128, 512], BF16, tag="t1", bufs=2,
                                         name="t1")
                            nc.vector.tensor_mul(t1[:], sh[:],
                                                 B_[:, lo:lo + 512])
                            t2 = bw.tile([128, 512], BF16, tag="t2", bufs=2,
                                         name="t2")
                            nc.vector.tensor_mul(t2[:], c[:],
                                                 A_[:, lo:lo + 512])
                            ro = roped[(b, mt)]
                            if is_q:
                                bc = bps.tile([128, 512], F32, tag=f"bc{mt}",
                                              name=f"bc{mt}")
                                nc.tensor.matmul(bc[:], ones_row[:],
                                                 rstd[mt][:], start=True,
                                                 stop=True)
                                bcb = bw.tile([128, 512], BF16,
                                              tag="bcb", bufs=2, name="bcb")
                                nc.scalar.copy(bcb[:], bc[:])
                                t3 = bw.tile([128, 512], BF16, tag="t3",
                                             bufs=2, name="t3")
                                nc.vector.tensor_add(t3[:], t1[:], t2[:])
                                nc.vector.tensor_mul(ro[:, lo:lo + 512],
                                                     t3[:], bcb[:])
                            else:
                                nc.vector.tensor_add(ro[:, lo:lo + 512],
                                                     t1[:], t2[:])

                    def finish_batch(b):
                        nc.sync.dma_start(rstdkT[b][:],
                                          rk_ds[b][:].rearrange("i p -> p i"))
                        if DEBUG and b == 0:
                            nc.sync.dma_start(dbg["roped00"].ap(),
                                              roped[(0, 0)][:])
                            nc.sync.dma_start(dbg["roped02"].ap(),
                                              roped[(0, 2)][:])
                            nc.sync.dma_start(dbg["vaug0"].ap(), vaug[0][:])
                            nc.sync.dma_start(dbg["rkt0"].ap(), rstdkT[0][:])

                    pairs = [(b, n) for b in range(B) for n in range(N_CHUNKS)]
                    prev_by_b = {0: None, 1: None}
                    pend_st2 = None  # (b, n, raws)
                    for (b, n) in pairs:
                        raws = stage1(n, b)
                        if (b, n) == (0, 0):
                            load_ropes()
                        if pend_st2 is not None:
                            pb, pn, praws = pend_st2
                            stage2(pn, praws, prev_by_b[pb], pb)
                            prev_by_b[pb] = praws
                            if pn == N_CHUNKS - 1:
                                finish_batch(pb)
                        pend_st2 = (b, n, raws)
                    pb, pn, praws = pend_st2
                    stage2(pn, praws, prev_by_b[pb], pb)
                    finish_batch(pb)

            # ======================= attention =======================
            # a2a buffers: one per local q-head, [8*128 rows, 512 cols]
            a2a_in = [dram.tile([N_CORES * 128, 512], BF16, tag=f"a2ai{h}",
                                name=f"a2a_in{h}") for h in range(2)]
            a2a_out = [dram.tile([N_CORES * 128, 512], BF16, tag=f"a2ao{h}",
                                 name=f"a2a_out{h}") for h in range(2)]

            with tc.tile_pool(name="scps", bufs=4, space="PSUM") as scps, \
                 tc.tile_pool(name="pvps", bufs=2, space="PSUM") as pvps, \
                 tc.tile_pool(name="rsps", bufs=1, space="PSUM") as rsps, \
                 tc.tile_pool(name="abps", bufs=1, space="PSUM") as abps, \
                 tc.tile_pool(name="apool", bufs=1) as apool:
                for h in range(2):
                    for b in range(B):
                        KT = roped[(b, 2)]
                        QT = roped[(b, h)]
                        va = vaug[b]
                        rkt = rstdkT[b]
                        for j in range(N_CHUNKS):
                            nij = 4 * j + 4
                            out_ps = pvps.tile([128, 512], F32, tag="pv",
                                               name="out_ps")
                            rs_ps = rsps.tile([1, 512], F32, tag="rs",
                                              name="rs_ps")

                            def emit_pv(i, pt):
                                nc.tensor.matmul(
                                    out_ps[:],
                                    va[:, 128 * i:128 * (i + 1)], pt[:],
                                    start=(i == 0), stop=(i == nij - 1))
                                nc.tensor.matmul(
                                    rs_ps[:], ones_col[:], pt[:],
                                    start=(i == 0), stop=(i == nij - 1))

                            pend = []
                            for i in range(nij):
                                r = i - 4 * j
                                off = 128 * max(r, 0)
                                sc = scps.tile([128, 512], F32, tag="sc",
                                               name="sc")
                                nc.tensor.matmul(
                                    sc[:, off:512],
                                    KT[:, 128 * i:128 * (i + 1)],
                                    QT[:, 512 * j + off:512 * (j + 1)],
                                    start=True, stop=True)
                                if r >= 0:
                                    nc.vector.tensor_add(
                                        sc[:, off:off + 128],
                                        sc[:, off:off + 128], mask_sb[:])
                                pt = apool.tile([128, 512], BF16, tag="pt",
                                                bufs=7, name="pt")
                                if off > 0:
                                    nc.vector.memset(pt[:, 0:off], 0.0)
                                nc.scalar.activation(
                                    pt[:, off:512], sc[:, off:512], AF.Exp,
                                    scale=rkt[:, i:i + 1])
                                if DEBUG and h == 0 and b == 0 and j == 0 and i == 0:
                                    nc.sync.dma_start(dbg["pt00"].ap(), pt[:])
                                pend.append((i, pt))
                                if len(pend) > 3:
                                    emit_pv(*pend.pop(0))
                            for it in pend:
                                emit_pv(*it)
                            # normalize and ship
                            rsb = apool.tile([1, 512], F32, tag="rsb",
                                             bufs=2, name="rsb")
                            nc.vector.tensor_copy(rsb[:], rs_ps[:])
                            rr = apool.tile([1, 512], F32R, tag="rr", bufs=2,
                                            name="rr")
                            with nc.allow_low_precision(
                                    reason="softmax denom f32r ample"):
                                nc.vector.reciprocal(rr[:], rsb[:])
                            bcn = abps.tile([128, 512], F32, tag="bcn",
                                            name="bcn")
                            nc.tensor.matmul(bcn[:], ones_row[:], rr[:],
                                             start=True, stop=True)
                            bcs = apool.tile([128, 512], F32, tag="bcs",
                                             bufs=2, name="bcs")
                            nc.vector.tensor_copy(bcs[:], bcn[:])
                            oa = apool.tile([128, 512], BF16, tag="oa",
                                            bufs=3, name="oa")
                            nc.vector.tensor_mul(oa[:], out_ps[:], bcs[:])
                            rd = 4 * b + j
                            if DEBUG and h == 0 and b == 0 and j == 0:
                                nc.sync.dma_start(dbg["oa00"].ap(), oa[:])
                            nc.sync.dma_start(
                                a2a_in[h][128 * rd:128 * (rd + 1), :], oa[:])
                    # per-head AllToAll as soon as this head's blocks done
                    nc.gpsimd.collective_compute(
                        "AllToAll", ALU.bypass,
                        replica_groups=[list(range(N_CORES))],
                        ins=[a2a_in[h].opt()], outs=[a2a_out[h].opt()],
                        cc_dim="Partition")

            # ====================== out projection ====================
            # channel block 2*s+h of the gathered activation lives in
            # a2a_out[h] src-block s.  Split the contraction by h so the
            # h=1 AllToAll hides behind the h=0 half of the matmul.
            with tc.tile_pool(name="opool", bufs=1) as opool, \
                 tc.tile_pool(name="ops", bufs=1, space="PSUM") as ops:
                aout = []
                for h in range(2):
                    blocks = []
                    for k8 in range(8):
                        abk = opool.tile([128, 512], BF16, tag=f"ao{h}_{k8}",
                                         name=f"ao{h}_{k8}")
                        nc.sync.dma_start(
                            abk[:],
                            a2a_out[h][128 * k8:128 * (k8 + 1), :]
                            .rearrange("p s -> p s"))
                        blocks.append(abk)
                    aout.append(blocks)
                if DEBUG:
                    nc.sync.dma_start(dbg["a2ao0"].ap(), a2a_out[0][:])
                for np_ in range(2):
                    pso = {}
                    for mp in range(4):
                        for nn in range(2):
                            pso[(mp, nn)] = ops.tile(
                                [128, 512], F32, tag=f"o{mp}{nn}",
                                name=f"o{mp}{nn}")
                    for h in range(2):
                        for k8 in range(8):
                            kb = 2 * k8 + h
                            wo_t = opool.tile([128, 1024], BF16, tag="wo",
                                              bufs=6, name="wo_t")
                            nc.sync.dma_start(
                                wo_t[:],
                                woT.ap()[128 * kb:128 * (kb + 1),
                                         1024 * np_:1024 * (np_ + 1)])
                            for mp in range(4):
                                for nn in range(2):
                                    nc.tensor.matmul(
                                        pso[(mp, nn)][:],
                                        aout[h][k8][:,
                                                    128 * mp:128 * (mp + 1)],
                                        wo_t[:, 512 * nn:512 * (nn + 1)],
                                        start=(h == 0 and k8 == 0),
                                        stop=(h == 1 and k8 == 7))
                    for mp in range(4):
                        for nn in range(2):
                            os_t = opool.tile([128, 512], F32, tag="osb",
                                              bufs=4, name="os_t")
                            nc.scalar.copy(os_t[:], pso[(mp, nn)][:])
                            nc.sync.dma_start(
                                out.ap()[128 * mp:128 * (mp + 1),
                                         1024 * np_ + 512 * nn:
                                         1024 * np_ + 512 * (nn + 1)],
                                os_t[:])

    nc.compile()
    return nc


_NC_CACHE = None


def _get_nc():
    global _NC_CACHE
    if _NC_CACHE is None:
        _NC_CACHE = _build()
    return _NC_CACHE


def _host_prep(inputs):
    hs = np.asarray(inputs["hidden_states"], dtype=np.float32)
    Wq = np.asarray(inputs["Wq"], dtype=np.float32)
    Wk = np.asarray(inputs["Wk"], dtype=np.float32)
    Wv = np.asarray(inputs["Wv"], dtype=np.float32)
    Wo = np.asarray(inputs["Wo"], dtype=np.float32)
    cqw = np.asarray(inputs["canon_q_w"], dtype=np.float32)
    ckw = np.asarray(inputs["canon_k_w"], dtype=np.float32)
    cvw = np.asarray(inputs["canon_v_w"], dtype=np.float32)
    qnw = np.asarray(inputs["q_norm_w"], dtype=np.float32)
    knw = np.asarray(inputs["k_norm_w"], dtype=np.float32)

    bf = ml_dtypes.bfloat16
    qkv_dt = ml_dtypes.float8_e4m3fn if FP8_QKV else bf
    hsT = np.ascontiguousarray(
        np.concatenate([hs[0].T, hs[1].T], axis=1)).astype(qkv_dt)
    WqT, WkT, WvT = Wq.T, Wk.T, Wv.T
    woT = np.ascontiguousarray(Wo.T).astype(bf)

    inv_freq = 1.0 / (10000.0 ** (np.arange(0, DH, 2, dtype=np.float64) / DH))
    freqs = np.arange(S, dtype=np.float64)[:, None] * inv_freq
    emb = np.concatenate([freqs, freqs], axis=-1)
    cosT, sinT = np.cos(emb).T, np.sin(emb).T

    def make_rope(normw, scale):
        A = cosT * normw[:, None] * scale
        wswap = normw[(np.arange(DH) + 64) % DH]
        sign = np.where(np.arange(DH) < 64, -1.0, 1.0)
        Bc = sinT * wswap[:, None] * sign[:, None] * scale
        return (np.ascontiguousarray(A).astype(bf),
                np.ascontiguousarray(Bc).astype(bf))

    Aq, Bq = make_rope(qnw, SCALE)
    Ak, Bk = make_rope(knw, 1.0)

    p = np.arange(128)[:, None]
    f = np.arange(128)[None, :]
    maskd = np.where(p <= f, 0.0, NEG).astype(np.float32)

    in_maps = []
    for r in range(N_CORES):
        wTc = np.ascontiguousarray(np.concatenate(
            [WqT[:, 256 * r:256 * r + 256],
             WkT[:, 128 * r:128 * r + 128],
             WvT[:, 128 * r:128 * r + 128]], axis=1)).astype(qkv_dt)
        cwc = np.ascontiguousarray(np.concatenate(
            [cqw[256 * r:256 * r + 256],
             ckw[128 * r:128 * r + 128],
             cvw[128 * r:128 * r + 128]], axis=0)).astype(np.float32)
        in_maps.append({
            "hsT": hsT, "wT": wTc, "woT": woT, "cw": cwc,
            "ropeAq": Aq, "ropeBq": Bq, "ropeAk": Ak, "ropeBk": Bk,
            "maskd": maskd,
        })
    return in_maps


def kernel(**inputs):
    nc = _get_nc()
    in_maps = _host_prep(inputs)
    res = run_bass_kernel_spmd(nc, in_maps, core_ids=list(range(N_CORES)))
    full = np.empty((B, S, D), np.float32)
    for r in range(N_CORES):
        full[r // 4, 512 * (r % 4):512 * (r % 4 + 1), :] = res.results[r]["out"]
    return full


# revision 17
# speedup vs baseline: 1.0513x; 1.0129x over previous
"""Trainium2 Bass kernel for CanonCausalMultiheadAttn (v2).

Sharding: tensor-parallel over heads across 8 cores (2 q-heads + 1 kv-head
per core), both batches replicated. Each core computes its heads' attention
for both batches; two AllToAlls (one per local q-head) exchange attention
outputs so each core owns one (batch, seq-slice) of the final output
projection.

v2 structure (vs v1):
  - Phase A fuses QKV proj + canon conv + qk-rmsnorm + rope per 512-seq
    chunk so DVE/Pool/Scalar work overlaps the PE matmuls and the PE never
    idles (keeps the PE p-state at full clock).  V's canon runs on the Pool
    (gpsimd) engine; x^2 and rsqrt run on the Scalar engine (single act
    table: {Copy, Square, Rsqrt}).
  - Phase B attention keeps scores in [sk, q] layout but computes PV as
    out[dh, q] = sum_i va_i^T @ P_i (va as stationary operand), which lands
    directly in the AllToAll layout - no per-block DMA transposes and no
    [128,1] reciprocals.  Row sums come from ones-column matmuls; the
    1/rowsum is broadcast via a rank-1 matmul and applied with one DVE mul.
    A lag-2 software pipeline hides the exp (Scalar) latency.
  - The AllToAll is split per local q-head: cc_h0 fires halfway through
    attention; cc_h1 is hidden behind the even-channel half of the output
    projection (split by channel-block parity).
"""
import sys

sys.path.insert(0, '/opt/trn_rl_repo')

import numpy as np
import ml_dtypes

import concourse.bass as bass
import concourse.mybir as mybir
import concourse.tile as tile
from concourse import bacc
from concourse.bass_utils import run_bass_kernel_spmd

F32 = mybir.dt.float32
F32R = mybir.dt.float32r
F8 = mybir.dt.float8e4
PM_DR = mybir.MatmulPerfMode.DoubleRow
BF16 = mybir.dt.bfloat16
AF = mybir.ActivationFunctionType
ALU = mybir.AluOpType

B, S, D = 2, 2048, 2048
NH, NKV, DH = 16, 8, 128
K_CONV = 4
EPS = 1e-6
SCALE = 1.0 / float(np.sqrt(DH))
NEG = -1e9
N_CORES = 8
N_CHUNKS = S // 512     # 512-wide seq chunks per batch
FP8_QKV = False         # fp8e4 DoubleRow QKV: fast but fails the 2e-2 gate
N_SKB = S // 128        # 128-wide sk blocks per batch


def _build():
    nc = bacc.Bacc("TRN2", target_bir_lowering=False, debug=False,
                   num_devices=N_CORES)

    QKV_DT = F8 if FP8_QKV else BF16
    hsT = nc.dram_tensor("hsT", [D, B * S], QKV_DT, kind="ExternalInput")
    wT = nc.dram_tensor("wT", [D, 512], QKV_DT, kind="ExternalInput")
    woT = nc.dram_tensor("woT", [D, D], BF16, kind="ExternalInput")
    cw = nc.dram_tensor("cw", [512, K_CONV], F32, kind="ExternalInput")
    ropeAq = nc.dram_tensor("ropeAq", [DH, S], BF16, kind="ExternalInput")
    ropeBq = nc.dram_tensor("ropeBq", [DH, S], BF16, kind="ExternalInput")
    ropeAk = nc.dram_tensor("ropeAk", [DH, S], BF16, kind="ExternalInput")
    ropeBk = nc.dram_tensor("ropeBk", [DH, S], BF16, kind="ExternalInput")
    maskd = nc.dram_tensor("maskd", [128, 128], F32, kind="ExternalInput")
    out = nc.dram_tensor("out", [512, D], F32, kind="ExternalOutput")
    import os
    DEBUG = os.environ.get("KDBG", "") == "1"
    dbg = {}
    if DEBUG:
        dbg["roped00"] = nc.dram_tensor("d_roped00", [128, S], BF16, kind="ExternalOutput")
        dbg["roped02"] = nc.dram_tensor("d_roped02", [128, S], BF16, kind="ExternalOutput")
        dbg["vaug0"] = nc.dram_tensor("d_vaug0", [128, N_SKB * 128], BF16, kind="ExternalOutput")
        dbg["rkt0"] = nc.dram_tensor("d_rkt0", [128, N_SKB], F32, kind="ExternalOutput")
        dbg["pt00"] = nc.dram_tensor("d_pt00", [128, 512], BF16, kind="ExternalOutput")
        dbg["rs00"] = nc.dram_tensor("d_rs00", [1, 512], F32, kind="ExternalOutput")
        dbg["oa00"] = nc.dram_tensor("d_oa00", [128, 512], BF16, kind="ExternalOutput")
        dbg["a2ao0"] = nc.dram_tensor("d_a2ao0", [N_CORES * 128, 512], BF16, kind="ExternalOutput")

    with tile.TileContext(nc) as tc:
        with tc.tile_pool(name="const", bufs=1) as cpool, \
             tc.tile_pool(name="persist", bufs=1) as pers, \
             tc.tile_pool(name="dram", bufs=1, space="DRAM") as dram:

            # ---- constants (weights first so QKV can start ASAP) ----
            wvq = []
            wsb_tiles = []
            for q4 in range(4):
                t = cpool.tile([128, 4 * 512], QKV_DT, tag=f"wsb{q4}",
                               name=f"wsb{q4}")
                wsb_tiles.append(t)
                wvq.append(t[:].rearrange("p (k s) -> p k s", s=512))

            def load_wsb(q4):
                nc.sync.dma_start(
                    wsb_tiles[q4][:].rearrange("p (k s) -> p k s", s=512),
                    wT.ap()[512 * q4:512 * (q4 + 1), :]
                    .rearrange("(k p) s -> p k s", p=128))
            load_wsb(0)
            mask_sb = cpool.tile([128, 128], F32, tag="mask")
            nc.sync.dma_start(mask_sb[:], maskd.ap())
            ropes = {}
            rope_dram = {"Aq": ropeAq, "Bq": ropeBq, "Ak": ropeAk,
                         "Bk": ropeBk}
            for nm in rope_dram:
                ropes[nm] = cpool.tile([DH, S], BF16, tag=f"rope{nm}",
                                       name=f"rope{nm}")

            def load_ropes():
                for nm, t in rope_dram.items():
                    nc.sync.dma_start(ropes[nm][:], t.ap())
            cw_sb = []
            for mt in range(4):
                t = cpool.tile([128, K_CONV], F32, tag=f"cw{mt}", name=f"cw{mt}")
                nc.sync.dma_start(t[:], cw.ap()[128 * mt:128 * mt + 128, :])
                cw_sb.append(t)
            ones_col = cpool.tile([128, 1], BF16, tag="oc")
            nc.vector.memset(ones_col[:], 1.0)
            eps_sb = cpool.tile([1, 1], F32, tag="eps")
            nc.vector.memset(eps_sb[:], EPS)
            ones_row_f = cpool.tile([1, 128], F32, tag="orf")
            nc.vector.memset(ones_row_f[:], 1.0)
            ones_row = cpool.tile([1, 128], F32R, tag="or")
            nc.scalar.copy(ones_row[:], ones_row_f[:])
            s0_sb = []
            for mt in range(4):
                t = cpool.tile([128, 1], F32, tag=f"s0{mt}", name=f"s0{mt}")
                nc.vector.tensor_scalar_add(t[:], cw_sb[mt][:, 0:1], 1.0)
                s0_sb.append(t)

            # persistent per-(b,mt) tiles
            roped = {}   # (b, mt<3) -> [128, S] bf16 (q0,q1 scaled by rstd)
            vaug = {}    # b -> [128, N_SKB*128] bf16 (transposed V)
            rstdkT = {}  # b -> [128, N_SKB] f32 (k rstd, transposed)

            for b in range(B):
                vaug[b] = pers.tile([128, N_SKB * 128], BF16, tag=f"vaug{b}",
                                    name=f"vaug{b}")
                rstdkT[b] = pers.tile([128, N_SKB], F32, tag=f"rstdkT{b}",
                                      name=f"rstdkT{b}")
                for mt in range(3):
                    roped[(b, mt)] = pers.tile([128, S], BF16,
                                               tag=f"roped{b}{mt}",
                                               name=f"roped{b}{mt}")

            # ============ phase A: QKV + canon + norm + rope ============
            # Two-stage pipeline: stage1(n) = hs DMA + QKV matmuls + psum->bf16
            # copies; stage2(n) = canon + norm + rope.  stage2(n-1) is emitted
            # after stage1(n) so its colsum/bcast matmuls never stall the PE.
            with tc.tile_pool(name="awork", bufs=1) as bw, \
                 tc.tile_pool(name="qps", bufs=1, space="PSUM") as qps, \
                 tc.tile_pool(name="nps", bufs=2, space="PSUM") as nps, \
                 tc.tile_pool(name="bps", bufs=1, space="PSUM") as bps:
                rk_ds = {b: dram.tile([N_SKB, 128], F32, tag=f"rkd{b}",
                                      name=f"rk_d{b}") for b in range(B)}
                if True:

                    def stage1(n, b):
                        rk_d = rk_ds[b]
                        lo = 512 * n
                        hsp = []
                        for q4 in range(4):
                            t = bw.tile([128, 4 * 512], QKV_DT,
                                        tag=f"hs{q4}", bufs=2,
                                        name=f"hs{q4}")
                            nc.sync.dma_start(
                                t[:].rearrange("p (k s) -> p k s", s=512),
                                hsT.ap()[512 * q4:512 * (q4 + 1),
                                         b * S + lo:b * S + lo + 512]
                                .rearrange("(k p) s -> p k s", p=128))
                            hsp.append(t[:].rearrange("p (k s) -> p k s",
                                                      s=512))
                        if (b, n) == (0, 0):
                            for q4 in range(1, 4):
                                load_wsb(q4)
                        psums = [qps.tile([128, 512], F32, tag=f"qk{mt}",
                                          name=f"qk{mt}") for mt in range(4)]
                        if FP8_QKV:
                            for k8 in range(8):
                                k = 2 * k8
                                for mt in range(4):
                                    nc.tensor.matmul(
                                        psums[mt][:],
                                        wvq[k // 4][:, k % 4:k % 4 + 2,
                                                    128 * mt:128 * (mt + 1)],
                                        hsp[k // 4][:, k % 4:k % 4 + 2, :],
                                        start=(k8 == 0), stop=(k8 == 7),
                                        perf_mode=PM_DR)
                        else:
                            for k in range(16):
                                for mt in range(4):
                                    nc.tensor.matmul(
                                        psums[mt][:],
                                        wvq[k // 4][:, k % 4,
                                                    128 * mt:128 * (mt + 1)],
                                        hsp[k // 4][:, k % 4, :],
                                        start=(k == 0), stop=(k == 15))
                        raws = []
                        for mt in range(4):
                            raw_c = bw.tile([128, 512], BF16, tag=f"rawc{mt}",
                                            bufs=3, name=f"rawc{mt}")
                            nc.scalar.copy(raw_c[:], psums[mt][:])
                            raws.append(raw_c)
                        return raws

                    def stage2(n, raws, prev, b):
                        rk_d = rk_ds[b]
                        lo = 512 * n
                        cn = {}
                        for mt in range(4):
                            c = bw.tile([128, 512], BF16, tag=f"cn{mt}",
                                        bufs=2, name=f"cn{mt}")
                            nc.vector.tensor_scalar_mul(c[:], raws[mt][:],
                                                        s0_sb[mt][:])
                            for k in range(1, K_CONV):
                                nc.vector.scalar_tensor_tensor(
                                    c[:, k:512], raws[mt][:, 0:512 - k],
                                    cw_sb[mt][:, k:k + 1], c[:, k:512],
                                    ALU.mult, ALU.add)
                                if prev is not None:
                                    nc.vector.scalar_tensor_tensor(
                                        c[:, 0:k],
                                        prev[mt][:, 512 - k:512],
                                        cw_sb[mt][:, k:k + 1], c[:, 0:k],
                                        ALU.mult, ALU.add)
                            cn[mt] = c
                        for i in range(4):
                            nc.sync.dma_start_transpose(
                                vaug[b][:, 128 * (4 * n + i):
                                        128 * (4 * n + i + 1)],
                                cn[3][:, 128 * i:128 * (i + 1)])
                        # rmsnorm rstd on the scalar engine:
                        # rstd = exp(-0.5*ln(meansq + eps))
                        rstd = {}
                        for mt in range(3):
                            sq = bw.tile([128, 512], BF16, tag="sq", bufs=2,
                                         name="sq")
                            nc.scalar.activation(sq[:], cn[mt][:], AF.Square)
                            sp = nps.tile([1, 512], F32, tag="ssq")
                            nc.tensor.matmul(sp[:], ones_col[:], sq[:],
                                             start=True, stop=True)
                            rs = bw.tile([1, 512], F32R if mt < 2 else F32,
                                         tag=f"rstd{mt}", bufs=2,
                                         name=f"rstd{mt}")
                            with nc.allow_low_precision(
                                    reason="rstd f32r is plenty"):
                                nc.scalar.activation(rs[:], sp[:],
                                                     AF.Abs_reciprocal_sqrt,
                                                     bias=eps_sb[:],
                                                     scale=1.0 / DH)
                            rstd[mt] = rs
                        nc.sync.dma_start(rk_d[4 * n:4 * (n + 1), :],
                                          rstd[2][:])
                        for mt in range(3):
                            is_q = mt < 2
                            A_ = ropes["Aq"] if is_q else ropes["Ak"]
                            B_ = ropes["Bq"] if is_q else ropes["Bk"]
                            c = cn[mt]
                            sh = bw.tile([128, 512], BF16, tag="sh", bufs=2,
                                         name="sh")
                            nc.sync.dma_start(sh[0:64, :], c[64:128, :])
                            nc.sync.dma_start(sh[64:128, :], c[0:64, :])
                            t1 = bw.tile([128, 512], BF16, tag="t1", bufs=2,
                                         name="t1")
                            nc.vector.tensor_mul(t1[:], sh[:],
                                                 B_[:, lo:lo + 512])
                            t2 = bw.tile([128, 512], BF16, tag="t2", bufs=2,
                                         name="t2")
                            nc.vector.tensor_mul(t2[:], c[:],
                                                 A_[:, lo:lo + 512])
                            ro = roped[(b, mt)]
                            if is_q:
                                bc = bps.tile([128, 512], F32, tag=f"bc{mt}",
                                              name=f"bc{mt}")
                                nc.tensor.matmul(bc[:], ones_row[:],
                                                 rstd[mt][:], start=True,
                                                 stop=True)
                                bcb = bw.tile([128, 512], BF16,
                                              tag="bcb", bufs=2, name="bcb")
                                nc.scalar.copy(bcb[:], bc[:])
                                t3 = bw.tile([128, 512], BF16, tag="t3",
                                             bufs=2, name="t3")
                                nc.vector.tensor_add(t3[:], t1[:], t2[:])
                                nc.vector.tensor_mul(ro[:, lo:lo + 512],
                                                     t3[:], bcb[:])
                            else:
                                nc.vector.tensor_add(ro[:, lo:lo + 512],
                                                     t1[:], t2[:])

                    def finish_batch(b):
                        nc.sync.dma_start(rstdkT[b][:],
                                          rk_ds[b][:].rearrange("i p -> p i"))
                        if DEBUG and b == 0:
                            nc.sync.dma_start(dbg["roped00"].ap(),
                                              roped[(0, 0)][:])
                            nc.sync.dma_start(dbg["roped02"].ap(),
                                              roped[(0, 2)][:])
                            nc.sync.dma_start(dbg["vaug0"].ap(), vaug[0][:])
                            nc.sync.dma_start(dbg["rkt0"].ap(), rstdkT[0][:])

                    pairs = [(b, n) for b in range(B) for n in range(N_CHUNKS)]
                    prev_by_b = {0: None, 1: None}
                    pend_st2 = None  # (b, n, raws)
                    for (b, n) in pairs:
                        raws = stage1(n, b)
                        if (b, n) == (0, 0):
                            load_ropes()
                        if pend_st2 is not None:
                            pb, pn, praws = pend_st2
                            stage2(pn, praws, prev_by_b[pb], pb)
                            prev_by_b[pb] = praws
                            if pn == N_CHUNKS - 1:
                                finish_batch(pb)
                        pend_st2 = (b, n, raws)
                    pb, pn, praws = pend_st2
                    stage2(pn, praws, prev_by_b[pb], pb)
                    finish_batch(pb)

            # ======================= attention =======================
            # a2a buffers: one per local q-head, [8*128 rows, 512 cols]
            a2a_in = [dram.tile([N_CORES * 128, 512], BF16, tag=f"a2ai{h}",
                                name=f"a2a_in{h}") for h in range(2)]
            a2a_out = [dram.tile([N_CORES * 128, 512], BF16, tag=f"a2ao{h}",
                                 name=f"a2a_out{h}") for h in range(2)]

            with tc.tile_pool(name="scps", bufs=4, space="PSUM") as scps, \
                 tc.tile_pool(name="pvps", bufs=2, space="PSUM") as pvps, \
                 tc.tile_pool(name="rsps", bufs=1, space="PSUM") as rsps, \
                 tc.tile_pool(name="abps", bufs=1, space="PSUM") as abps, \
                 tc.tile_pool(name="apool", bufs=1) as apool:
                for h in range(2):
                    for b in range(B):
                        KT = roped[(b, 2)]
                        QT = roped[(b, h)]
                        va = vaug[b]
                        rkt = rstdkT[b]
                        for j in range(N_CHUNKS):
                            nij = 4 * j + 4
                            out_ps = pvps.tile([128, 512], F32, tag="pv",
                                               name="out_ps")
                            rs_ps = rsps.tile([1, 512], F32, tag="rs",
                                              name="rs_ps")

                            def emit_pv(i, pt):
                                nc.tensor.matmul(
                                    out_ps[:],
                                    va[:, 128 * i:128 * (i + 1)], pt[:],
                                    start=(i == 0), stop=(i == nij - 1))
                                nc.tensor.matmul(
                                    rs_ps[:], ones_col[:], pt[:],
                                    start=(i == 0), stop=(i == nij - 1))

                            pend = []
                            for i in range(nij):
                                r = i - 4 * j
                                off = 128 * max(r, 0)
                                sc = scps.tile([128, 512], F32, tag="sc",
                                               name="sc")
                                nc.tensor.matmul(
                                    sc[:, off:512],
                                    KT[:, 128 * i:128 * (i + 1)],
                                    QT[:, 512 * j + off:512 * (j + 1)],
                                    start=True, stop=True)
                                if r >= 0:
                                    nc.vector.tensor_add(
                                        sc[:, off:off + 128],
                                        sc[:, off:off + 128], mask_sb[:])
                                pt = apool.tile([128, 512], BF16, tag="pt",
                                                bufs=7, name="pt")
                                if off > 0:
                                    nc.vector.memset(pt[:, 0:off], 0.0)
                                nc.scalar.activation(
                                    pt[:, off:512], sc[:, off:512], AF.Exp,
                                    scale=rkt[:, i:i + 1])
                                if DEBUG and h == 0 and b == 0 and j == 0 and i == 0:
                                    nc.sync.dma_start(dbg["pt00"].ap(), pt[:])
                                pend.append((i, pt))
                                if len(pend) > 3:
                                    emit_pv(*pend.pop(0))
                            for it in pend:
                                emit_pv(*it)
                            # normalize and ship
                            rsb = apool.tile([1, 512], F32, tag="rsb",
                                             bufs=2, name="rsb")
                            nc.vector.tensor_copy(rsb[:], rs_ps[:])
                            rr = apool.tile([1, 512], F32R, tag="rr", bufs=2,
                                            name="rr")
                            with nc.allow_low_precision(
                                    reason="softmax denom f32r ample"):
                                nc.vector.reciprocal(rr[:], rsb[:])
                            bcn = abps.tile([128, 512], F32, tag="bcn",
                                            name="bcn")
                            nc.tensor.matmul(bcn[:], ones_row[:], rr[:],
                                             start=True, stop=True)
                            bcs = apool.tile([128, 512], F32, tag="bcs",
                                             bufs=2, name="bcs")
                            nc.vector.tensor_copy(bcs[:], bcn[:])
                            oa = apool.tile([128, 512], BF16, tag="oa",
                                            bufs=3, name="oa")
                            nc.vector.tensor_mul(oa[:], out_ps[:], bcs[:])
                            rd = 4 * b + j
                            if DEBUG and h == 0 and b == 0 and j == 0:
                                nc.sync.dma_start(dbg["oa00"].ap(), oa[:])
                            nc.sync.dma_start(
                                a2a_in[h][128 * rd:128 * (rd + 1), :], oa[:])
                    # per-head AllToAll as soon as this head's blocks done
                    nc.gpsimd.collective_compute(
                        "AllToAll", ALU.bypass,
                        replica_groups=[list(range(N_CORES))],
                        ins=[a2a_in[h].opt()], outs=[a2a_out[h].opt()],
                        cc_dim="Partition")

            # ====================== out projection ====================
            # channel block 2*s+h of the gathered activation lives in
            # a2a_out[h] src-block s.  Split the contraction by h so the
            # h=1 AllToAll hides behind the h=0 half of the matmul.
            with tc.tile_pool(name="opool", bufs=1) as opool, \
                 tc.tile_pool(name="ops", bufs=1, space="PSUM") as ops:
                aout = []
                for h in range(2):
                    blocks = []
                    for k8 in range(8):
                        abk = opool.tile([128, 512], BF16, tag=f"ao{h}_{k8}",
                                         name=f"ao{h}_{k8}")
                        nc.sync.dma_start(
                            abk[:],
                            a2a_out[h][128 * k8:128 * (k8 + 1), :]
                            .rearrange("p s -> p s"))
                        blocks.append(abk)
                    aout.append(blocks)
                if DEBUG:
                    nc.sync.dma_start(dbg["a2ao0"].ap(), a2a_out[0][:])
                for np_ in range(2):
                    pso = {}
                    for mp in range(4):
                        for nn in range(2):
                            pso[(mp, nn)] = ops.tile(
                                [128, 512], F32, tag=f"o{mp}{nn}",
                                name=f"o{mp}{nn}")
                    for h in range(2):
                        for k8 in range(8):
                            kb = 2 * k8 + h
                            wo_t = opool.tile([128, 1024], BF16, tag="wo",
                                              bufs=6, name="wo_t")
                            nc.sync.dma_start(
                                wo_t[:],
                                woT.ap()[128 * kb:128 * (kb + 1),
                                         1024 * np_:1024 * (np_ + 1)])
                            for mp in range(4):
                                for nn in range(2):
                                    nc.tensor.matmul(
                                        pso[(mp, nn)][:],
                                        aout[h][k8][:,
                                                    128 * mp:128 * (mp + 1)],
                                        wo_t[:, 512 * nn:512 * (nn + 1)],
                                        start=(h == 0 and k8 == 0),
                                        stop=(h == 1 and k8 == 7))
                    for mp in range(4):
                        for nn in range(2):
                            os_t = opool.tile([128, 512], F32, tag="osb",
                                              bufs=4, name="os_t")
                            nc.scalar.copy(os_t[:], pso[(mp, nn)][:])
                            nc.sync.dma_start(
                                out.ap()[128 * mp:128 * (mp + 1),
                                         1024 * np_ + 512 * nn:
                                         1024 * np_ + 512 * (nn + 1)],
                                os_t[:])

    nc.compile()
    return nc


_NC_CACHE = None


def _get_nc():
    global _NC_CACHE
    if _NC_CACHE is None:
        _NC_CACHE = _build()
    return _NC_CACHE


def _host_prep(inputs):
    hs = np.asarray(inputs["hidden_states"], dtype=np.float32)
    Wq = np.asarray(inputs["Wq"], dtype=np.float32)
    Wk = np.asarray(inputs["Wk"], dtype=np.float32)
    Wv = np.asarray(inputs["Wv"], dtype=np.float32)
    Wo = np.asarray(inputs["Wo"], dtype=np.float32)
    cqw = np.asarray(inputs["canon_q_w"], dtype=np.float32)
    ckw = np.asarray(inputs["canon_k_w"], dtype=np.float32)
    cvw = np.asarray(inputs["canon_v_w"], dtype=np.float32)
    qnw = np.asarray(inputs["q_norm_w"], dtype=np.float32)
    knw = np.asarray(inputs["k_norm_w"], dtype=np.float32)

    bf = ml_dtypes.bfloat16
    qkv_dt = ml_dtypes.float8_e4m3fn if FP8_QKV else bf
    hsT = np.ascontiguousarray(
        np.concatenate([hs[0].T, hs[1].T], axis=1)).astype(qkv_dt)
    WqT, WkT, WvT = Wq.T, Wk.T, Wv.T
    woT = np.ascontiguousarray(Wo.T).astype(bf)

    inv_freq = 1.0 / (10000.0 ** (np.arange(0, DH, 2, dtype=np.float64) / DH))
    freqs = np.arange(S, dtype=np.float64)[:, None] * inv_freq
    emb = np.concatenate([freqs, freqs], axis=-1)
    cosT, sinT = np.cos(emb).T, np.sin(emb).T

    def make_rope(normw, scale):
        A = cosT * normw[:, None] * scale
        wswap = normw[(np.arange(DH) + 64) % DH]
        sign = np.where(np.arange(DH) < 64, -1.0, 1.0)
        Bc = sinT * wswap[:, None] * sign[:, None] * scale
        return (np.ascontiguousarray(A).astype(bf),
                np.ascontiguousarray(Bc).astype(bf))

    Aq, Bq = make_rope(qnw, SCALE)
    Ak, Bk = make_rope(knw, 1.0)

    p = np.arange(128)[:, None]
    f = np.arange(128)[None, :]
    maskd = np.where(p <= f, 0.0, NEG).astype(np.float32)

    in_maps = []
    for r in range(N_CORES):
        wTc = np.ascontiguousarray(np.concatenate(
            [WqT[:, 256 * r:256 * r + 256],
             WkT[:, 128 * r:128 * r + 128],
             WvT[:, 128 * r:128 * r + 128]], axis=1)).astype(qkv_dt)
        cwc = np.ascontiguousarray(np.concatenate(
            [cqw[256 * r:256 * r + 256],
             ckw[128 * r:128 * r + 128],
             cvw[128 * r:128 * r + 128]], axis=0)).astype(np.float32)
        in_maps.append({
            "hsT": hsT, "wT": wTc, "woT": woT, "cw": cwc,
            "ropeAq": Aq, "ropeBq": Bq, "ropeAk": Ak, "ropeBk": Bk,
            "maskd": maskd,
        })
    return in_maps


def kernel(**inputs):
    nc = _get_nc()
    in_maps = _host_prep(inputs)
    res = run_bass_kernel_spmd(nc, in_maps, core_ids=list(range(N_CORES)))
    full = np.empty((B, S, D), np.float32)
    for r in range(N_CORES):
        full[r // 4, 512 * (r % 4):512 * (r % 4 + 1), :] = res.results[r]["out"]
    return full
